# revision 12
# baseline (speedup 1.0000x reference)
"""Trainium2 Bass kernel for nn_CellDecoder (span-pool + ffnn + biaffine pairs).

Strategy: head_idx/tail_idx only reference E=256 entities, so instead of
computing the biaffine per pair (P=65536), the cores build the full E x E
biaffine logit table (small matmuls). The per-pair work is a pure table
lookup with host-known indices, done during the host-side unshard/assembly
step, so the device kernel ships the dense table.

Sharding: 8 cores = batch (2) x e1-half (2) x e2-half (2). Each core
computes one 128x128 quadrant of the logit table (for both output logits):
the head ffnn chain runs on its 128 row-entities and the tail chain on its
128 column-entities. Per-core "which entities" is steered purely through
the inputs (SPMD program identical on all cores).

Key algebraic optimization vs the earlier version: the label-embedding
half of layer 1 is folded on the host:
    ent_repr @ W1 = pooled @ W1[:D] + emb[label] @ W1[D:]
and emb @ W1[D:] is a weights-only product -> precompute C = emb_w @ W1b
([5, H1] per side) on host; apply on device as a K=5 one-hot matmul that
rides the same PSUM accumulation as layer 1. This halves W1 DMA bytes
(2.36MB), removes the embedding DMA (0.46MB), and halves layer-1 tensor
cycles.

Perf notes:
- Everything is bf16; rel err ~5e-3, well under the 2e-2 gate.
- DMA issue is spread over four engine rings (scalar/sync/vector/gpsimd)
  in exact consumption order so the tensor engine starts pooling as soon
  as mask + first hs chunk land, and never starves afterwards.
- Matmul cost on TRN2 is ~(128 LDWEIGHTS + N moving) cycles; the kernel
  keeps N as large as the sharding allows.
- psum->sbuf copies alternate vector/scalar engines.
"""

import os

os.environ.setdefault("JAX_PLATFORMS", "axon,cpu")

import numpy as np
import ml_dtypes

import concourse.bass as bass
import concourse.tile as tile
from concourse import bacc, mybir
from concourse.bass_utils import run_bass_kernel_spmd

dt = mybir.dt

B, T, D, E, P = 2, 512, 768, 256, 65536
MLP = 2 * D  # 1536
H1, H2 = MLP // 2, MLP // 4  # 768, 384
NL = 5
OUT = 2
N_CORES = 8
EH = 128  # table rows/cols per core (quadrant edge)

KT_D = D // 128  # 6   (layer-1 contraction tiles over pooled part only)
KT_H1 = H1 // 128  # 6
KT_H2 = H2 // 128  # 3
KT_T = T // 128  # 4
MT_D = D // 128  # 6
MT_H1 = H1 // 128  # 6
MT_H2 = H2 // 128  # 3

SF_COLS = 2 * MT_H1 + 2 * MT_H2 + OUT  # b1h, b1t, b2h, b2t, blin = 20
PKG_COLS = 2 * H1 + E + E  # C_h | C_t | onehot | ones-block

_cache: dict = {}


def _build(ni: int = 0):
    """Build + compile the SPMD program (ni unused, kept for test.py interface)."""
    if 0 in _cache:
        return _cache[0]

    nc = bacc.Bacc("TRN2", target_bir_lowering=False, debug=False, num_devices=N_CORES)

    f32 = dt.float32
    bf16 = dt.bfloat16

    # [128, cols] host-packed operand tensors
    d_hs = nc.dram_tensor("hs", [128, KT_T * D], bf16, kind="ExternalInput")
    d_maskn = nc.dram_tensor("masknT", [128, KT_T * E], bf16, kind="ExternalInput")
    d_w1h = nc.dram_tensor("W1h", [128, KT_D * H1], bf16, kind="ExternalInput")
    d_w1t = nc.dram_tensor("W1t", [128, KT_D * H1], bf16, kind="ExternalInput")
    d_w2h = nc.dram_tensor("W2h", [128, KT_H1 * H2], bf16, kind="ExternalInput")
    d_w2t = nc.dram_tensor("W2t", [128, KT_H1 * H2], bf16, kind="ExternalInput")
    # Wbil_0 | Wbil_1 | wlin (head/tail cols for both o)
    d_wtl = nc.dram_tensor(
        "Wtl", [128, 2 * KT_H2 * H2 + 2 * KT_H2 * OUT], bf16, kind="ExternalInput"
    )
    # [5, .]: C_h | C_t | one-hot labels | ones-block (row 0 = 1.0)
    d_pkg = nc.dram_tensor("pkg", [NL, PKG_COLS], bf16, kind="ExternalInput")
    d_smf = nc.dram_tensor("smf", [128, SF_COLS], f32, kind="ExternalInput")
    # output: this core's table quadrant per o, columns in subset order
    d_slab = nc.dram_tensor("slab", [128, OUT * EH], bf16, kind="ExternalOutput")

    with tile.TileContext(nc) as tc:
        with (
            tc.tile_pool(name="wbig", bufs=1) as wbig,
            tc.tile_pool(name="wsml", bufs=1) as wsml,
            tc.tile_pool(name="act", bufs=1) as act,
            tc.tile_pool(name="ps", bufs=4, space="PSUM") as ps,
            tc.tile_pool(name="ps1", bufs=2, space="PSUM") as ps1,
            tc.tile_pool(name="ps2", bufs=2, space="PSUM") as ps2,
        ):
            # ---- DMA issue in consumption order ----
            # scalar ring: maskn (2 chunks so pooling kt=0 starts early),
            # then the small stuff
            maskn = wsml.tile([128, KT_T, E], bf16, tag="maskn", name="maskn")
            maskn_src = d_maskn.ap().rearrange("p (kt n) -> p kt n", kt=KT_T)
            nc.scalar.dma_start(maskn[:, 0:2, :], maskn_src[:, 0:2, :])
            nc.scalar.dma_start(maskn[:, 2:4, :], maskn_src[:, 2:4, :])
            smf = wsml.tile([128, SF_COLS], f32, tag="smf", name="smf")
            nc.scalar.dma_start(smf[:], d_smf.ap())
            pkg = wsml.tile([NL, PKG_COLS], bf16, tag="pkg", name="pkg")
            nc.scalar.dma_start(pkg[:], d_pkg.ap())

            # sync ring: ALL bulk tensors, in exact consumption order.
            # (Do NOT spread bulk across rings: DMA engines round-robin
            # across queued descriptors, so multiple bulk rings destroy the
            # arrival ordering and starve the tensor engine mid-kernel.)
            hs = wbig.tile([128, KT_T, D], bf16, tag="hs", name="hs")
            hs_src = d_hs.ap().rearrange("p (kt n) -> p kt n", kt=KT_T)
            for kt in range(KT_T):
                nc.sync.dma_start(hs[:, kt : kt + 1, :], hs_src[:, kt : kt + 1, :])

            w1 = {}
            w2 = {}
            w1["h"] = wbig.tile([128, KT_D, H1], bf16, tag="w1h", name="w1h")
            w1h_src = d_w1h.ap().rearrange("p (kt n) -> p kt n", kt=KT_D)
            nc.sync.dma_start(w1["h"][:, 0:3, :], w1h_src[:, 0:3, :])
            nc.sync.dma_start(w1["h"][:, 3:6, :], w1h_src[:, 3:6, :])
            w2["h"] = wbig.tile([128, KT_H1, H2], bf16, tag="w2h", name="w2h")
            nc.sync.dma_start(
                w2["h"][:], d_w2h.ap().rearrange("p (kt n) -> p kt n", kt=KT_H1)
            )

            w1["t"] = wbig.tile([128, KT_D, H1], bf16, tag="w1t", name="w1t")
            w1t_src = d_w1t.ap().rearrange("p (kt n) -> p kt n", kt=KT_D)
            nc.sync.dma_start(w1["t"][:, 0:3, :], w1t_src[:, 0:3, :])
            nc.sync.dma_start(w1["t"][:, 3:6, :], w1t_src[:, 3:6, :])
            w2["t"] = wbig.tile([128, KT_H1, H2], bf16, tag="w2t", name="w2t")
            nc.sync.dma_start(
                w2["t"][:], d_w2t.ap().rearrange("p (kt n) -> p kt n", kt=KT_H1)
            )
            wtl = wsml.tile([128, KT_H2, 2 * H2 + 2 * OUT], bf16, tag="wtl", name="wtl")
            nc.sync.dma_start(
                wtl[:, :, 0 : 2 * H2],
                d_wtl.ap()[:, 0 : 2 * KT_H2 * H2].rearrange(
                    "p (kt n) -> p kt n", kt=KT_H2
                ),
            )
            nc.sync.dma_start(
                wtl[:, :, 2 * H2 :],
                d_wtl.ap()[:, 2 * KT_H2 * H2 :].rearrange(
                    "p (kt n) -> p kt n", kt=KT_H2
                ),
            )
            wb = [wtl[:, :, 0:H2], wtl[:, :, H2 : 2 * H2]]
            # per o: [:, kt, 2*H2 + 2*o] = head col, [:, kt, 2*H2 + 2*o + 1] = tail col

            # views into pkg
            cmat = pkg[:, 0 : 2 * H1].rearrange("p (s n) -> p s n", s=2)
            onehot = pkg[:, 2 * H1 : 2 * H1 + E]
            ones_t = pkg[0:1, 2 * H1 + E : 2 * H1 + 2 * E]

            b1 = {"h": smf[:, 0:MT_H1], "t": smf[:, MT_H1 : 2 * MT_H1]}
            b2 = {
                "h": smf[:, 2 * MT_H1 : 2 * MT_H1 + MT_H2],
                "t": smf[:, 2 * MT_H1 + MT_H2 : 2 * MT_H1 + 2 * MT_H2],
            }
            blin = smf[0:1, 2 * MT_H1 + 2 * MT_H2 : SF_COLS]

            # copy engines alternate to halve serial copy chains
            def copy(i, dst, src):
                if i % 2:
                    nc.scalar.activation(
                        dst, src, mybir.ActivationFunctionType.Identity
                    )
                else:
                    nc.vector.tensor_copy(dst, src)

            # ---- pooled^T -> entT  (two passes of 3 mt-tiles to fit PSUM;
            # pass A streams with arriving hs chunks, pass B runs after) ----
            entT = act.tile([128, KT_D, E], bf16, tag="entT")
            for half in range(2):
                pool_ps = [
                    ps.tile([128, E], f32, tag="mm", name=f"pp{half}{m}")
                    for m in range(3)
                ]
                for kt in range(KT_T):
                    for m in range(3):
                        mt = half * 3 + m
                        nc.tensor.matmul(
                            pool_ps[m][:],
                            hs[:, kt, mt * 128 : (mt + 1) * 128],
                            maskn[:, kt, :],
                            start=(kt == 0),
                            stop=(kt == KT_T - 1),
                        )
                for m in range(3):
                    copy(m, entT[:, half * 3 + m, :], pool_ps[m][:])

            # ---- ffnn chains on 128-entity subsets:
            #      head = cols 0:128, tail = cols 128:256 ----
            # Weights-moving form: stationary = activation tile (one
            # LDWEIGHTS per contraction tile), moving = weight columns
            # (N=384) -> ~2x fewer LDWEIGHTS cycles than weight-stationary.
            # Output lands entity-major [E, H]; the [H, E] layout needed by
            # the next stage comes from a DMA XBAR transpose (zero tensor
            # cycles, runs on the idle gpsimd ring).
            h2T = {}

            def ffnn(side):
                si = 0 if side == "h" else 1
                lo = si * EH
                # layer 1: psum [ents, 768] split as 2 x [128, 384]
                pl1 = [
                    ps2.tile([128, H1 // 2], f32, tag="pl1", name=f"l1{side}{j}")
                    for j in range(2)
                ]
                for kt in range(KT_D):
                    for j in range(2):
                        nc.tensor.matmul(
                            pl1[j][:],
                            entT[:, kt, lo : lo + EH],
                            w1[side][:, kt, j * 384 : (j + 1) * 384],
                            start=(kt == 0),
                            stop=False,
                        )
                # label-embedding contribution: K=5 one-hot matmul, C on host
                for j in range(2):
                    nc.tensor.matmul(
                        pl1[j][:],
                        onehot[:, lo : lo + EH],
                        cmat[:, si, j * 384 : (j + 1) * 384],
                        start=False,
                        stop=True,
                    )
                # evacuate raw (bias+relu only valid after transpose)
                h1raw = act.tile(
                    [128, 2, 384], bf16, tag=f"h1raw{side}", name=f"h1raw{side}"
                )
                for j in range(2):
                    copy(j, h1raw[:, j, :], pl1[j][:])
                # XBAR transpose [ents, 768] -> [768, ents] as [128, 6, 128]
                h1Tr = act.tile(
                    [128, KT_H1, EH], bf16, tag=f"h1Tr{side}", name=f"h1Tr{side}"
                )
                for j in range(2):
                    nc.scalar.dma_start_transpose(
                        h1Tr[:, 3 * j : 3 * j + 3, :], h1raw[:, j, :]
                    )
                h1T = act.tile(
                    [128, KT_H1, EH], bf16, tag=f"h1T{side}", name=f"h1T{side}"
                )
                for mt in range(MT_H1):
                    nc.scalar.activation(
                        h1T[:, mt, :],
                        h1Tr[:, mt, :],
                        mybir.ActivationFunctionType.Relu,
                        bias=b1[side][:, mt : mt + 1],
                    )
                # layer 2: psum [ents, 384]
                pl2 = ps2.tile([128, H2], f32, tag="pl1", name=f"l2{side}")
                for kt in range(KT_H1):
                    nc.tensor.matmul(
                        pl2[:],
                        h1T[:, kt, :],
                        w2[side][:, kt, :],
                        start=(kt == 0),
                        stop=(kt == KT_H1 - 1),
                    )
                h2raw = act.tile([128, H2], bf16, tag=f"h2raw{side}", name=f"h2raw{side}")
                copy(si, h2raw[:], pl2[:])
                h2Tr = act.tile(
                    [128, KT_H2, EH], bf16, tag=f"h2Tr{side}", name=f"h2Tr{side}"
                )
                nc.scalar.dma_start_transpose(h2Tr[:], h2raw[:])
                h2T[side] = act.tile(
                    [128, KT_H2, EH], bf16, tag=f"h2T{side}", name=f"h2T{side}"
                )
                for mt in range(MT_H2):
                    nc.scalar.activation(
                        h2T[side][:, mt, :],
                        h2Tr[:, mt, :],
                        mybir.ActivationFunctionType.Relu,
                        bias=b2[side][:, mt : mt + 1],
                    )

            ffnn("h")

            # ---- N_o^T [H2, EH] and linh [1, EH] for both o ----
            nT = []
            linh = []
            for o in range(OUT):
                nTo = act.tile([128, KT_H2, EH], bf16, tag=f"nT{o}", name=f"nT{o}")
                accs = [
                    ps.tile([128, EH], f32, tag="mm", name=f"nt{o}{m}")
                    for m in range(MT_H2)
                ]
                for kt in range(KT_H2):
                    for mt in range(MT_H2):
                        nc.tensor.matmul(
                            accs[mt][:],
                            wb[o][:, kt, mt * 128 : (mt + 1) * 128],
                            h2T["h"][:, kt, :],
                            start=(kt == 0),
                            stop=(kt == KT_H2 - 1),
                        )
                for mt in range(MT_H2):
                    copy(mt, nTo[:, mt, :], accs[mt][:])
                nT.append(nTo)

                lh = act.tile([1, EH], bf16, tag=f"linh{o}", name=f"linh{o}")
                p = ps1.tile([1, EH], f32, tag="lin")
                for kt in range(KT_H2):
                    nc.tensor.matmul(
                        p[:],
                        wtl[:, kt, 2 * H2 + 2 * o : 2 * H2 + 2 * o + 1],
                        h2T["h"][:, kt, :],
                        start=(kt == 0),
                        stop=(kt == KT_H2 - 1),
                    )
                nc.vector.tensor_copy(lh[:], p[:])
                linh.append(lh)

            ffnn("t")

            lint = []
            for o in range(OUT):
                lt = act.tile([1, EH], bf16, tag=f"lint{o}", name=f"lint{o}")
                p = ps1.tile([1, EH], f32, tag="lin")
                for kt in range(KT_H2):
                    nc.tensor.matmul(
                        p[:],
                        wtl[:, kt, 2 * H2 + 2 * o + 1 : 2 * H2 + 2 * o + 2],
                        h2T["t"][:, kt, :],
                        start=(kt == 0),
                        stop=(kt == KT_H2 - 1),
                    )
                # + b_lin[o] folded in via bias
                nc.scalar.activation(
                    lt[:],
                    p[:],
                    mybir.ActivationFunctionType.Identity,
                    bias=blin[:, o : o + 1],
                )
                lint.append(lt)

            # ---- table quadrant for this core: [128, OUT, EH] ----
            slab = act.tile([128, OUT, EH], bf16, tag="slab")
            slab_dst = d_slab.ap().rearrange("p (o n) -> p o n", o=OUT)
            for o in range(OUT):
                p = ps.tile([128, EH], f32, tag="mm")
                for kt in range(KT_H2):
                    nc.tensor.matmul(
                        p[:],
                        nT[o][:, kt, :],
                        h2T["t"][:, kt, :],
                        start=(kt == 0),
                        stop=False,
                    )
                nc.tensor.matmul(
                    p[:], linh[o][:], ones_t[:, 0:EH], start=False, stop=False
                )
                nc.tensor.matmul(
                    p[:], ones_t[:, 0:128], lint[o][:], start=False, stop=True
                )
                copy(o, slab[:, o, :], p[:])
                # ship each o-slab as soon as it's ready
                nc.sync.dma_start(slab_dst[:, o : o + 1, :], slab[:, o : o + 1, :])

    nc.compile()
    _cache[0] = nc
    return nc


def _pack(w, kt, dtype=ml_dtypes.bfloat16):
    """[kt*128, n] row-major -> [128, kt*n] partition-packed."""
    n = w.shape[1]
    return np.ascontiguousarray(
        w.reshape(kt, 128, n).transpose(1, 0, 2).reshape(128, kt * n).astype(dtype)
    )


def _prep_host(inputs):
    """Host-side input packing -> per-core in_maps + assembly info."""
    hs = np.asarray(inputs["hidden_states"], dtype=np.float32)
    start = np.asarray(inputs["entity_start"]).astype(np.int64)
    end = np.asarray(inputs["entity_end"]).astype(np.int64)
    label = np.asarray(inputs["entity_label"]).astype(np.int64)

    t = np.arange(T)
    mask = (
        (t[None, None, :] >= start[:, :, None]) & (t[None, None, :] < end[:, :, None])
    ).astype(np.float32)  # [B,E,T]
    counts = np.maximum(mask.sum(-1, keepdims=True), 1.0)
    masknT = (mask / counts).transpose(0, 2, 1)  # [B,T,E]

    def f32(x):
        return np.ascontiguousarray(np.asarray(x, dtype=np.float32))

    bf = ml_dtypes.bfloat16
    w_bil = f32(inputs["W_bil"])
    w_lin = f32(inputs["W_lin"])
    b_lin = f32(inputs["b_lin"])
    emb_all = f32(inputs["entity_emb_w"])

    # Wbil_0 | Wbil_1 | wlin cols interleaved per o as [head_o, tail_o]
    wb0 = _pack(w_bil[0], KT_H2, np.float32).reshape(128, KT_H2, H2)
    wb1 = _pack(w_bil[1], KT_H2, np.float32).reshape(128, KT_H2, H2)
    wl = np.stack(
        [
            w_lin[:H2, 0].reshape(KT_H2, 128).T,
            w_lin[H2:, 0].reshape(KT_H2, 128).T,
            w_lin[:H2, 1].reshape(KT_H2, 128).T,
            w_lin[H2:, 1].reshape(KT_H2, 128).T,
        ],
        axis=2,
    )  # [128, KT_H2, 4]
    region1 = np.concatenate([wb0, wb1], axis=2).reshape(128, -1)
    region2 = wl.reshape(128, -1)
    wtl = np.ascontiguousarray(
        np.concatenate([region1, region2], axis=1).astype(bf)
    )

    smf = np.zeros((128, SF_COLS), np.float32)
    smf[:, 0:MT_H1] = f32(inputs["bh1"]).reshape(MT_H1, 128).T
    smf[:, MT_H1 : 2 * MT_H1] = f32(inputs["bt1"]).reshape(MT_H1, 128).T
    smf[:, 2 * MT_H1 : 2 * MT_H1 + MT_H2] = f32(inputs["bh2"]).reshape(MT_H2, 128).T
    smf[:, 2 * MT_H1 + MT_H2 : 2 * MT_H1 + 2 * MT_H2] = (
        f32(inputs["bt2"]).reshape(MT_H2, 128).T
    )
    smf[0, 2 * MT_H1 + 2 * MT_H2 : SF_COLS] = b_lin

    # layer-1 split: W1a = W1[:D] on device; C = emb @ W1[D:] folded on host
    w1a = {}
    cfold = {}
    for s, key in (("h", "Wh1"), ("t", "Wt1")):
        w1f = f32(inputs[key])
        w1a[s] = _pack(w1f[:D], KT_D)
        cfold[s] = (emb_all @ w1f[D:]).astype(np.float32)  # [NL, H1]

    shared = {
        "W1h": w1a["h"],
        "W1t": w1a["t"],
        "W2h": _pack(f32(inputs["Wh2"]), KT_H1),
        "W2t": _pack(f32(inputs["Wt2"]), KT_H1),
        "Wtl": wtl,
        "smf": smf,
    }

    in_maps = []
    for i in range(N_CORES):
        b, q = divmod(i, 4)
        r, c = divmod(q, 2)  # row-half, col-half of the table quadrant
        sel = np.concatenate(
            [np.arange(EH * r, EH * r + EH), np.arange(EH * c, EH * c + EH)]
        )  # [head subset | tail subset]
        mrot = np.ascontiguousarray(masknT[b][:, sel])
        lab = label[b][sel]  # [E]
        onehot = (lab[None, :] == np.arange(NL)[:, None]).astype(np.float32)  # [NL,E]
        pkg = np.zeros((NL, PKG_COLS), np.float32)
        pkg[:, 0:H1] = cfold["h"]
        pkg[:, H1 : 2 * H1] = cfold["t"]
        pkg[:, 2 * H1 : 2 * H1 + E] = onehot
        pkg[0, 2 * H1 + E :] = 1.0  # ones row
        mm = dict(shared)
        mm["hs"] = _pack(hs[b], KT_T)
        mm["masknT"] = _pack(mrot, KT_T)
        mm["pkg"] = pkg.astype(bf)
        in_maps.append(mm)

    head_idx = np.asarray(inputs["head_idx"]).astype(np.int64)
    tail_idx = np.asarray(inputs["tail_idx"]).astype(np.int64)
    return in_maps, (head_idx, tail_idx), 0


def kernel(**inputs) -> np.ndarray:
    in_maps, (head_idx, tail_idx), ni = _prep_host(inputs)
    nc = _build(ni)
    res = run_bass_kernel_spmd(nc, in_maps, list(range(N_CORES)))
    out = np.zeros((B, P, OUT), np.float32)
    for b in range(B):
        slabs = np.stack(
            [
                res.results[4 * b + q]["slab"].reshape(128, OUT, EH).astype(np.float32)
                for q in range(4)
            ]
        )  # [q, 128, OUT, EH]; q = 2*r + c
        e1, e2 = head_idx[b], tail_idx[b]
        q = 2 * (e1 // EH) + (e2 // EH)
        out[b] = slabs[q, e1 % EH, :, e2 % EH]
    return out


# revision 16
# speedup vs baseline: 1.2602x; 1.2602x over previous
"""Trainium2 Bass kernel for nn_CellDecoder (span-pool + ffnn + biaffine pairs).

Strategy: head_idx/tail_idx only reference E=256 entities, so instead of
computing the biaffine per pair (P=65536), the cores build the full E x E
biaffine logit table (small matmuls). The per-pair work is a pure table
lookup with host-known indices, done during the host-side unshard/assembly
step, so the device kernel ships the dense table.

Sharding: 8 cores = batch (2) x e1-half (2) x e2-half (2). Each core
computes one 128x128 quadrant of the logit table (for both output logits):
the head ffnn chain runs on its 128 row-entities and the tail chain on its
128 column-entities. Per-core "which entities" is steered purely through
the inputs (SPMD program identical on all cores).

Key algebraic optimization vs the earlier version: the label-embedding
half of layer 1 is folded on the host:
    ent_repr @ W1 = pooled @ W1[:D] + emb[label] @ W1[D:]
and emb @ W1[D:] is a weights-only product -> precompute C = emb_w @ W1b
([5, H1] per side) on host; apply on device as a K=5 one-hot matmul that
rides the same PSUM accumulation as layer 1. This halves W1 DMA bytes
(2.36MB), removes the embedding DMA (0.46MB), and halves layer-1 tensor
cycles.

Perf notes:
- Everything is bf16; rel err ~5e-3, well under the 2e-2 gate.
- DMA issue is spread over four engine rings (scalar/sync/vector/gpsimd)
  in exact consumption order so the tensor engine starts pooling as soon
  as mask + first hs chunk land, and never starves afterwards.
- Matmul cost on TRN2 is ~(128 LDWEIGHTS + N moving) cycles; the kernel
  keeps N as large as the sharding allows.
- psum->sbuf copies alternate vector/scalar engines.
"""

import os

os.environ.setdefault("JAX_PLATFORMS", "axon,cpu")

import numpy as np
import ml_dtypes

import concourse.bass as bass
import concourse.tile as tile
from concourse import bacc, mybir
from concourse.bass_utils import run_bass_kernel_spmd

dt = mybir.dt

B, T, D, E, P = 2, 512, 768, 256, 65536
MLP = 2 * D  # 1536
H1, H2 = MLP // 2, MLP // 4  # 768, 384
NL = 5
OUT = 2
N_CORES = 8
EH = 128  # table rows/cols per core (quadrant edge)

KT_D = D // 128  # 6   (layer-1 contraction tiles over pooled part only)
KT_H1 = H1 // 128  # 6
KT_H2 = H2 // 128  # 3
KT_T = T // 128  # 4
MT_D = D // 128  # 6
MT_H1 = H1 // 128  # 6
MT_H2 = H2 // 128  # 3

SF_COLS = 2 * MT_H1 + 2 * MT_H2 + OUT  # b1h, b1t, b2h, b2t, blin = 20
PKG_COLS = 2 * H1 + E + E  # C_h | C_t | onehot | ones-block

_cache: dict = {}


def _build(ni: int = 0):
    """Build + compile the SPMD program (ni unused, kept for test.py interface)."""
    if 0 in _cache:
        return _cache[0]

    nc = bacc.Bacc("TRN2", target_bir_lowering=False, debug=False, num_devices=N_CORES)

    f32 = dt.float32
    bf16 = dt.bfloat16

    # [128, cols] host-packed operand tensors
    d_hs = nc.dram_tensor("hs", [128, KT_T * D], bf16, kind="ExternalInput")
    d_maskn = nc.dram_tensor("masknT", [128, KT_T * E], bf16, kind="ExternalInput")
    d_w1h = nc.dram_tensor("W1h", [128, KT_D * H1], bf16, kind="ExternalInput")
    d_w1t = nc.dram_tensor("W1t", [128, KT_D * H1], bf16, kind="ExternalInput")
    d_w2h = nc.dram_tensor("W2h", [128, KT_H1 * H2], bf16, kind="ExternalInput")
    d_w2t = nc.dram_tensor("W2t", [128, KT_H1 * H2], bf16, kind="ExternalInput")
    # Wbil_0 | Wbil_1 | wlin (head/tail cols for both o)
    d_wtl = nc.dram_tensor(
        "Wtl", [128, 2 * KT_H2 * H2 + 2 * KT_H2 * OUT], bf16, kind="ExternalInput"
    )
    # [5, .]: C_h | C_t | one-hot labels | ones-block (row 0 = 1.0)
    d_pkg = nc.dram_tensor("pkg", [NL, PKG_COLS], bf16, kind="ExternalInput")
    d_smf = nc.dram_tensor("smf", [128, SF_COLS], f32, kind="ExternalInput")
    # output: this core's table quadrant per o, columns in subset order
    d_slab = nc.dram_tensor("slab", [128, OUT * EH], bf16, kind="ExternalOutput")

    with tile.TileContext(nc) as tc:
        with (
            tc.tile_pool(name="wbig", bufs=1) as wbig,
            tc.tile_pool(name="wsml", bufs=1) as wsml,
            tc.tile_pool(name="act", bufs=1) as act,
            tc.tile_pool(name="ps", bufs=6, space="PSUM") as ps,
            tc.tile_pool(name="ps1", bufs=2, space="PSUM") as ps1,
        ):
            # ---- PE p-state warm-up ----
            # The tensor engine clocks up only after ~3us of continuous
            # work; burn junk matmuls on a zeroed tile during the DMA wait
            # so the real matmuls start at full clock.
            warm = wsml.tile([128, 512], bf16, tag="warm", name="warm")
            nc.gpsimd.memset(warm[:], 0)
            wp = ps1.tile([1, 512], f32, tag="lin", name="warmp")
            for i in range(6):
                nc.tensor.matmul(
                    wp[:], warm[:, 0:1], warm[:], start=(i == 0), stop=(i == 5)
                )

            # ---- DMA issue in consumption order ----
            # scalar ring: maskn (2 chunks so pooling kt=0 starts early),
            # then the small stuff
            maskn = wsml.tile([128, KT_T, E], bf16, tag="maskn", name="maskn")
            maskn_src = d_maskn.ap().rearrange("p (kt n) -> p kt n", kt=KT_T)
            nc.scalar.dma_start(maskn[:, 0:2, :], maskn_src[:, 0:2, :])
            nc.scalar.dma_start(maskn[:, 2:4, :], maskn_src[:, 2:4, :])
            smf = wsml.tile([128, SF_COLS], f32, tag="smf", name="smf")
            nc.scalar.dma_start(smf[:], d_smf.ap())
            pkg = wsml.tile([NL, PKG_COLS], bf16, tag="pkg", name="pkg")
            nc.scalar.dma_start(pkg[:], d_pkg.ap())

            # sync ring: ALL bulk tensors, in exact consumption order.
            # (Do NOT spread bulk across rings: DMA engines round-robin
            # across queued descriptors, so multiple bulk rings destroy the
            # arrival ordering and starve the tensor engine mid-kernel.)
            hs = wbig.tile([128, KT_T, D], bf16, tag="hs", name="hs")
            hs_src = d_hs.ap().rearrange("p (kt n) -> p kt n", kt=KT_T)
            for kt in range(KT_T):
                nc.sync.dma_start(hs[:, kt : kt + 1, :], hs_src[:, kt : kt + 1, :])

            w1 = {}
            w2 = {}
            w1["h"] = wbig.tile([128, KT_D, H1], bf16, tag="w1h", name="w1h")
            w1h_src = d_w1h.ap().rearrange("p (kt n) -> p kt n", kt=KT_D)
            nc.sync.dma_start(w1["h"][:, 0:3, :], w1h_src[:, 0:3, :])
            nc.sync.dma_start(w1["h"][:, 3:6, :], w1h_src[:, 3:6, :])
            w2["h"] = wbig.tile([128, KT_H1, H2], bf16, tag="w2h", name="w2h")
            nc.sync.dma_start(
                w2["h"][:], d_w2h.ap().rearrange("p (kt n) -> p kt n", kt=KT_H1)
            )

            w1["t"] = wbig.tile([128, KT_D, H1], bf16, tag="w1t", name="w1t")
            w1t_src = d_w1t.ap().rearrange("p (kt n) -> p kt n", kt=KT_D)
            nc.sync.dma_start(w1["t"][:, 0:3, :], w1t_src[:, 0:3, :])
            nc.sync.dma_start(w1["t"][:, 3:6, :], w1t_src[:, 3:6, :])
            w2["t"] = wbig.tile([128, KT_H1, H2], bf16, tag="w2t", name="w2t")
            nc.sync.dma_start(
                w2["t"][:], d_w2t.ap().rearrange("p (kt n) -> p kt n", kt=KT_H1)
            )
            wtl = wsml.tile([128, KT_H2, 2 * H2 + 2 * OUT], bf16, tag="wtl", name="wtl")
            nc.sync.dma_start(
                wtl[:, :, 0 : 2 * H2],
                d_wtl.ap()[:, 0 : 2 * KT_H2 * H2].rearrange(
                    "p (kt n) -> p kt n", kt=KT_H2
                ),
            )
            nc.sync.dma_start(
                wtl[:, :, 2 * H2 :],
                d_wtl.ap()[:, 2 * KT_H2 * H2 :].rearrange(
                    "p (kt n) -> p kt n", kt=KT_H2
                ),
            )
            wb = [wtl[:, :, 0:H2], wtl[:, :, H2 : 2 * H2]]
            # per o: [:, kt, 2*H2 + 2*o] = head col, [:, kt, 2*H2 + 2*o + 1] = tail col

            # views into pkg
            cmat = pkg[:, 0 : 2 * H1].rearrange("p (s n) -> p s n", s=2)
            onehot = pkg[:, 2 * H1 : 2 * H1 + E]
            ones_t = pkg[0:1, 2 * H1 + E : 2 * H1 + 2 * E]

            b1 = {"h": smf[:, 0:MT_H1], "t": smf[:, MT_H1 : 2 * MT_H1]}
            b2 = {
                "h": smf[:, 2 * MT_H1 : 2 * MT_H1 + MT_H2],
                "t": smf[:, 2 * MT_H1 + MT_H2 : 2 * MT_H1 + 2 * MT_H2],
            }
            blin = smf[0:1, 2 * MT_H1 + 2 * MT_H2 : SF_COLS]

            # copy engines alternate to halve serial copy chains
            def copy(i, dst, src):
                if i % 2:
                    nc.scalar.activation(
                        dst, src, mybir.ActivationFunctionType.Identity
                    )
                else:
                    nc.vector.tensor_copy(dst, src)

            # ---- pooled^T -> entT  (kt-outer over hs chunks) ----
            entT = act.tile([128, KT_D, E], bf16, tag="entT")
            pool_ps = [
                ps.tile([128, E], f32, tag="mm", name=f"pp{m}") for m in range(MT_D)
            ]
            for kt in range(KT_T):
                for mt in range(MT_D):
                    nc.tensor.matmul(
                        pool_ps[mt][:],
                        hs[:, kt, mt * 128 : (mt + 1) * 128],
                        maskn[:, kt, :],
                        start=(kt == 0),
                        stop=(kt == KT_T - 1),
                    )
            for mt in range(MT_D):
                copy(mt, entT[:, mt, :], pool_ps[mt][:])

            # ---- ffnn chains on 128-entity subsets:
            #      head = cols 0:128, tail = cols 128:256 ----
            h2T = {}

            def ffnn(side):
                si = 0 if side == "h" else 1
                lo = si * EH
                h1T = act.tile(
                    [128, KT_H1, EH], bf16, tag=f"h1T{side}", name=f"h1T{side}"
                )
                accs = [
                    ps.tile([128, EH], f32, tag="mm", name=f"l1{side}{m}")
                    for m in range(MT_H1)
                ]
                for kt in range(KT_D):
                    for mt in range(MT_H1):
                        nc.tensor.matmul(
                            accs[mt][:],
                            w1[side][:, kt, mt * 128 : (mt + 1) * 128],
                            entT[:, kt, lo : lo + EH],
                            start=(kt == 0),
                            stop=False,
                        )
                # label-embedding contribution: K=5 one-hot matmul, C on host
                for mt in range(MT_H1):
                    nc.tensor.matmul(
                        accs[mt][:],
                        cmat[:, si, mt * 128 : (mt + 1) * 128],
                        onehot[:, lo : lo + EH],
                        start=False,
                        stop=True,
                    )
                for mt in range(MT_H1):
                    nc.scalar.activation(
                        h1T[:, mt, :],
                        accs[mt][:],
                        mybir.ActivationFunctionType.Relu,
                        bias=b1[side][:, mt : mt + 1],
                    )
                h2T[side] = act.tile(
                    [128, KT_H2, EH], bf16, tag=f"h2T{side}", name=f"h2T{side}"
                )
                accs2 = [
                    ps.tile([128, EH], f32, tag="mm", name=f"l2{side}{m}")
                    for m in range(MT_H2)
                ]
                for kt in range(KT_H1):
                    for mt in range(MT_H2):
                        nc.tensor.matmul(
                            accs2[mt][:],
                            w2[side][:, kt, mt * 128 : (mt + 1) * 128],
                            h1T[:, kt, :],
                            start=(kt == 0),
                            stop=(kt == KT_H1 - 1),
                        )
                for mt in range(MT_H2):
                    nc.scalar.activation(
                        h2T[side][:, mt, :],
                        accs2[mt][:],
                        mybir.ActivationFunctionType.Relu,
                        bias=b2[side][:, mt : mt + 1],
                    )

            ffnn("h")

            # ---- N_o^T [H2, EH] and linh [1, EH] for both o ----
            nT = []
            linh = []
            for o in range(OUT):
                nTo = act.tile([128, KT_H2, EH], bf16, tag=f"nT{o}", name=f"nT{o}")
                accs = [
                    ps.tile([128, EH], f32, tag="mm", name=f"nt{o}{m}")
                    for m in range(MT_H2)
                ]
                for kt in range(KT_H2):
                    for mt in range(MT_H2):
                        nc.tensor.matmul(
                            accs[mt][:],
                            wb[o][:, kt, mt * 128 : (mt + 1) * 128],
                            h2T["h"][:, kt, :],
                            start=(kt == 0),
                            stop=(kt == KT_H2 - 1),
                        )
                for mt in range(MT_H2):
                    copy(mt, nTo[:, mt, :], accs[mt][:])
                nT.append(nTo)

                lh = act.tile([1, EH], bf16, tag=f"linh{o}", name=f"linh{o}")
                p = ps1.tile([1, EH], f32, tag="lin")
                for kt in range(KT_H2):
                    nc.tensor.matmul(
                        p[:],
                        wtl[:, kt, 2 * H2 + 2 * o : 2 * H2 + 2 * o + 1],
                        h2T["h"][:, kt, :],
                        start=(kt == 0),
                        stop=(kt == KT_H2 - 1),
                    )
                nc.vector.tensor_copy(lh[:], p[:])
                linh.append(lh)

            ffnn("t")

            lint = []
            for o in range(OUT):
                lt = act.tile([1, EH], bf16, tag=f"lint{o}", name=f"lint{o}")
                p = ps1.tile([1, EH], f32, tag="lin")
                for kt in range(KT_H2):
                    nc.tensor.matmul(
                        p[:],
                        wtl[:, kt, 2 * H2 + 2 * o + 1 : 2 * H2 + 2 * o + 2],
                        h2T["t"][:, kt, :],
                        start=(kt == 0),
                        stop=(kt == KT_H2 - 1),
                    )
                # + b_lin[o] folded in via bias
                nc.scalar.activation(
                    lt[:],
                    p[:],
                    mybir.ActivationFunctionType.Identity,
                    bias=blin[:, o : o + 1],
                )
                lint.append(lt)

            # ---- table quadrant for this core: [128, OUT, EH] ----
            slab = act.tile([128, OUT, EH], bf16, tag="slab")
            slab_dst = d_slab.ap().rearrange("p (o n) -> p o n", o=OUT)
            for o in range(OUT):
                p = ps.tile([128, EH], f32, tag="mm")
                for kt in range(KT_H2):
                    nc.tensor.matmul(
                        p[:],
                        nT[o][:, kt, :],
                        h2T["t"][:, kt, :],
                        start=(kt == 0),
                        stop=False,
                    )
                nc.tensor.matmul(
                    p[:], linh[o][:], ones_t[:, 0:EH], start=False, stop=False
                )
                nc.tensor.matmul(
                    p[:], ones_t[:, 0:128], lint[o][:], start=False, stop=True
                )
                copy(o, slab[:, o, :], p[:])
                # ship each o-slab as soon as it's ready
                nc.sync.dma_start(slab_dst[:, o : o + 1, :], slab[:, o : o + 1, :])

    nc.compile()
    _cache[0] = nc
    return nc


def _pack(w, kt, dtype=ml_dtypes.bfloat16):
    """[kt*128, n] row-major -> [128, kt*n] partition-packed."""
    n = w.shape[1]
    return np.ascontiguousarray(
        w.reshape(kt, 128, n).transpose(1, 0, 2).reshape(128, kt * n).astype(dtype)
    )


def _prep_host(inputs):
    """Host-side input packing -> per-core in_maps + assembly info."""
    hs = np.asarray(inputs["hidden_states"], dtype=np.float32)
    start = np.asarray(inputs["entity_start"]).astype(np.int64)
    end = np.asarray(inputs["entity_end"]).astype(np.int64)
    label = np.asarray(inputs["entity_label"]).astype(np.int64)

    t = np.arange(T)
    mask = (
        (t[None, None, :] >= start[:, :, None]) & (t[None, None, :] < end[:, :, None])
    ).astype(np.float32)  # [B,E,T]
    counts = np.maximum(mask.sum(-1, keepdims=True), 1.0)
    masknT = (mask / counts).transpose(0, 2, 1)  # [B,T,E]

    def f32(x):
        return np.ascontiguousarray(np.asarray(x, dtype=np.float32))

    bf = ml_dtypes.bfloat16
    w_bil = f32(inputs["W_bil"])
    w_lin = f32(inputs["W_lin"])
    b_lin = f32(inputs["b_lin"])
    emb_all = f32(inputs["entity_emb_w"])

    # Wbil_0 | Wbil_1 | wlin cols interleaved per o as [head_o, tail_o]
    wb0 = _pack(w_bil[0], KT_H2, np.float32).reshape(128, KT_H2, H2)
    wb1 = _pack(w_bil[1], KT_H2, np.float32).reshape(128, KT_H2, H2)
    wl = np.stack(
        [
            w_lin[:H2, 0].reshape(KT_H2, 128).T,
            w_lin[H2:, 0].reshape(KT_H2, 128).T,
            w_lin[:H2, 1].reshape(KT_H2, 128).T,
            w_lin[H2:, 1].reshape(KT_H2, 128).T,
        ],
        axis=2,
    )  # [128, KT_H2, 4]
    region1 = np.concatenate([wb0, wb1], axis=2).reshape(128, -1)
    region2 = wl.reshape(128, -1)
    wtl = np.ascontiguousarray(
        np.concatenate([region1, region2], axis=1).astype(bf)
    )

    smf = np.zeros((128, SF_COLS), np.float32)
    smf[:, 0:MT_H1] = f32(inputs["bh1"]).reshape(MT_H1, 128).T
    smf[:, MT_H1 : 2 * MT_H1] = f32(inputs["bt1"]).reshape(MT_H1, 128).T
    smf[:, 2 * MT_H1 : 2 * MT_H1 + MT_H2] = f32(inputs["bh2"]).reshape(MT_H2, 128).T
    smf[:, 2 * MT_H1 + MT_H2 : 2 * MT_H1 + 2 * MT_H2] = (
        f32(inputs["bt2"]).reshape(MT_H2, 128).T
    )
    smf[0, 2 * MT_H1 + 2 * MT_H2 : SF_COLS] = b_lin

    # layer-1 split: W1a = W1[:D] on device; C = emb @ W1[D:] folded on host
    w1a = {}
    cfold = {}
    for s, key in (("h", "Wh1"), ("t", "Wt1")):
        w1f = f32(inputs[key])
        w1a[s] = _pack(w1f[:D], KT_D)
        cfold[s] = (emb_all @ w1f[D:]).astype(np.float32)  # [NL, H1]

    shared = {
        "W1h": w1a["h"],
        "W1t": w1a["t"],
        "W2h": _pack(f32(inputs["Wh2"]), KT_H1),
        "W2t": _pack(f32(inputs["Wt2"]), KT_H1),
        "Wtl": wtl,
        "smf": smf,
    }

    in_maps = []
    for i in range(N_CORES):
        b, q = divmod(i, 4)
        r, c = divmod(q, 2)  # row-half, col-half of the table quadrant
        sel = np.concatenate(
            [np.arange(EH * r, EH * r + EH), np.arange(EH * c, EH * c + EH)]
        )  # [head subset | tail subset]
        mrot = np.ascontiguousarray(masknT[b][:, sel])
        lab = label[b][sel]  # [E]
        onehot = (lab[None, :] == np.arange(NL)[:, None]).astype(np.float32)  # [NL,E]
        pkg = np.zeros((NL, PKG_COLS), np.float32)
        pkg[:, 0:H1] = cfold["h"]
        pkg[:, H1 : 2 * H1] = cfold["t"]
        pkg[:, 2 * H1 : 2 * H1 + E] = onehot
        pkg[0, 2 * H1 + E :] = 1.0  # ones row
        mm = dict(shared)
        mm["hs"] = _pack(hs[b], KT_T)
        mm["masknT"] = _pack(mrot, KT_T)
        mm["pkg"] = pkg.astype(bf)
        in_maps.append(mm)

    head_idx = np.asarray(inputs["head_idx"]).astype(np.int64)
    tail_idx = np.asarray(inputs["tail_idx"]).astype(np.int64)
    return in_maps, (head_idx, tail_idx), 0


def kernel(**inputs) -> np.ndarray:
    in_maps, (head_idx, tail_idx), ni = _prep_host(inputs)
    nc = _build(ni)
    res = run_bass_kernel_spmd(nc, in_maps, list(range(N_CORES)))
    out = np.zeros((B, P, OUT), np.float32)
    for b in range(B):
        slabs = np.stack(
            [
                res.results[4 * b + q]["slab"].reshape(128, OUT, EH).astype(np.float32)
                for q in range(4)
            ]
        )  # [q, 128, OUT, EH]; q = 2*r + c
        e1, e2 = head_idx[b], tail_idx[b]
        q = 2 * (e1 // EH) + (e2 // EH)
        out[b] = slabs[q, e1 % EH, :, e2 % EH]
    return out


# revision 18
# speedup vs baseline: 1.4549x; 1.1545x over previous
"""Trainium2 Bass kernel for nn_CellDecoder (span-pool + ffnn + biaffine pairs).

Strategy: head_idx/tail_idx only reference E=256 entities, so instead of
computing the biaffine per pair (P=65536), the cores build the full E x E
biaffine logit table (small matmuls). The per-pair work is a pure table
lookup with host-known indices, done during the host-side unshard/assembly
step, so the device kernel ships the dense table.

Sharding: 8 cores = batch (2) x e1-half (2) x e2-half (2). Each core
computes one 128x128 quadrant of the logit table (for both output logits):
the head ffnn chain runs on its 128 row-entities and the tail chain on its
128 column-entities. Per-core "which entities" is steered purely through
the inputs (SPMD program identical on all cores).

Key algebraic optimization vs the earlier version: the label-embedding
half of layer 1 is folded on the host:
    ent_repr @ W1 = pooled @ W1[:D] + emb[label] @ W1[D:]
and emb @ W1[D:] is a weights-only product -> precompute C = emb_w @ W1b
([5, H1] per side) on host; apply on device as a K=5 one-hot matmul that
rides the same PSUM accumulation as layer 1. This halves W1 DMA bytes
(2.36MB), removes the embedding DMA (0.46MB), and halves layer-1 tensor
cycles.

Perf notes:
- Everything is bf16; rel err ~5e-3, well under the 2e-2 gate.
- DMA issue is spread over four engine rings (scalar/sync/vector/gpsimd)
  in exact consumption order so the tensor engine starts pooling as soon
  as mask + first hs chunk land, and never starves afterwards.
- Matmul cost on TRN2 is ~(128 LDWEIGHTS + N moving) cycles; the kernel
  keeps N as large as the sharding allows.
- psum->sbuf copies alternate vector/scalar engines.
"""

import os

os.environ.setdefault("JAX_PLATFORMS", "axon,cpu")

import numpy as np
import ml_dtypes

import concourse.bass as bass
import concourse.tile as tile
from concourse import bacc, mybir
from concourse.bass_utils import run_bass_kernel_spmd

dt = mybir.dt

B, T, D, E, P = 2, 512, 768, 256, 65536
MLP = 2 * D  # 1536
H1, H2 = MLP // 2, MLP // 4  # 768, 384
NL = 5
OUT = 2
N_CORES = 8
EH = 128  # table rows/cols per core (quadrant edge)

KT_D = D // 128  # 6   (layer-1 contraction tiles over pooled part only)
KT_H1 = H1 // 128  # 6
KT_H2 = H2 // 128  # 3
KT_T = T // 128  # 4
MT_D = D // 128  # 6
MT_H1 = H1 // 128  # 6
MT_H2 = H2 // 128  # 3

SF_COLS = 2 * MT_H1 + 2 * MT_H2 + OUT  # b1h, b1t, b2h, b2t, blin = 20
PKG_COLS = 2 * H1 + E + E  # C_h | C_t | onehot | ones-block

_cache: dict = {}


def _build(ni: int = 0):
    """Build + compile the SPMD program (ni unused, kept for test.py interface)."""
    if 0 in _cache:
        return _cache[0]

    nc = bacc.Bacc("TRN2", target_bir_lowering=False, debug=False, num_devices=N_CORES)

    f32 = dt.float32
    bf16 = dt.bfloat16

    # [128, cols] host-packed operand tensors
    d_hs = nc.dram_tensor("hs", [128, KT_T * D], bf16, kind="ExternalInput")
    d_maskn = nc.dram_tensor("masknT", [128, KT_T * E], bf16, kind="ExternalInput")
    d_w1h = nc.dram_tensor("W1h", [128, KT_D * H1], bf16, kind="ExternalInput")
    d_w1t = nc.dram_tensor("W1t", [128, KT_D * H1], bf16, kind="ExternalInput")
    d_w2h = nc.dram_tensor("W2h", [128, KT_H1 * H2], bf16, kind="ExternalInput")
    d_w2t = nc.dram_tensor("W2t", [128, KT_H1 * H2], bf16, kind="ExternalInput")
    # Wbil_0 | Wbil_1 | wlin (head/tail cols for both o)
    d_wtl = nc.dram_tensor(
        "Wtl", [128, 2 * KT_H2 * H2 + 2 * KT_H2 * OUT], bf16, kind="ExternalInput"
    )
    # [5, .]: C_h | C_t | one-hot labels | ones-block (row 0 = 1.0)
    d_pkg = nc.dram_tensor("pkg", [NL, PKG_COLS], bf16, kind="ExternalInput")
    d_smf = nc.dram_tensor("smf", [128, SF_COLS], f32, kind="ExternalInput")
    # output: this core's table quadrant per o, columns in subset order
    d_slab = nc.dram_tensor("slab", [128, OUT * EH], bf16, kind="ExternalOutput")

    with tile.TileContext(nc) as tc:
        with (
            tc.tile_pool(name="wbig", bufs=1) as wbig,
            tc.tile_pool(name="wsml", bufs=1) as wsml,
            tc.tile_pool(name="act", bufs=1) as act,
            tc.tile_pool(name="ps", bufs=6, space="PSUM") as ps,
            tc.tile_pool(name="ps1", bufs=2, space="PSUM") as ps1,
        ):
            # ---- DMA issue in consumption order ----
            # (No PE warm-up: TRN2's power governor throttles the tensor
            # engine to 50% util in proportion to activity — junk matmuls
            # increase throttle time and net-lose.)
            # scalar ring: maskn (2 chunks so pooling kt=0 starts early),
            # then the small stuff
            maskn = wsml.tile([128, KT_T, E], bf16, tag="maskn", name="maskn")
            maskn_src = d_maskn.ap().rearrange("p (kt n) -> p kt n", kt=KT_T)
            nc.scalar.dma_start(maskn[:, 0:2, :], maskn_src[:, 0:2, :])
            nc.scalar.dma_start(maskn[:, 2:4, :], maskn_src[:, 2:4, :])
            smf = wsml.tile([128, SF_COLS], f32, tag="smf", name="smf")
            nc.scalar.dma_start(smf[:], d_smf.ap())
            pkg = wsml.tile([NL, PKG_COLS], bf16, tag="pkg", name="pkg")
            nc.scalar.dma_start(pkg[:], d_pkg.ap())

            # sync ring: ALL bulk tensors, in exact consumption order.
            # (Do NOT spread bulk across rings: DMA engines round-robin
            # across queued descriptors, so multiple bulk rings destroy the
            # arrival ordering and starve the tensor engine mid-kernel.)
            hs = wbig.tile([128, KT_T, D], bf16, tag="hs", name="hs")
            hs_src = d_hs.ap().rearrange("p (kt n) -> p kt n", kt=KT_T)
            for kt in range(KT_T):
                nc.sync.dma_start(hs[:, kt : kt + 1, :], hs_src[:, kt : kt + 1, :])

            w1 = {}
            w2 = {}
            w1["h"] = wbig.tile([128, KT_D, H1], bf16, tag="w1h", name="w1h")
            w1h_src = d_w1h.ap().rearrange("p (kt n) -> p kt n", kt=KT_D)
            nc.sync.dma_start(w1["h"][:, 0:3, :], w1h_src[:, 0:3, :])
            nc.sync.dma_start(w1["h"][:, 3:6, :], w1h_src[:, 3:6, :])
            w2["h"] = wbig.tile([128, KT_H1, H2], bf16, tag="w2h", name="w2h")
            nc.sync.dma_start(
                w2["h"][:], d_w2h.ap().rearrange("p (kt n) -> p kt n", kt=KT_H1)
            )

            # wtl ships BEFORE w1t/w2t: the tensor queue is in-order and the
            # nT matmuls (which need wtl) sit between ffnn-h and ffnn-t in
            # program order — wtl arriving last would stall the whole queue.
            wtl = wsml.tile([128, KT_H2, 2 * H2 + 2 * OUT], bf16, tag="wtl", name="wtl")
            nc.sync.dma_start(
                wtl[:, :, 0 : 2 * H2],
                d_wtl.ap()[:, 0 : 2 * KT_H2 * H2].rearrange(
                    "p (kt n) -> p kt n", kt=KT_H2
                ),
            )
            nc.sync.dma_start(
                wtl[:, :, 2 * H2 :],
                d_wtl.ap()[:, 2 * KT_H2 * H2 :].rearrange(
                    "p (kt n) -> p kt n", kt=KT_H2
                ),
            )
            w1["t"] = wbig.tile([128, KT_D, H1], bf16, tag="w1t", name="w1t")
            w1t_src = d_w1t.ap().rearrange("p (kt n) -> p kt n", kt=KT_D)
            nc.sync.dma_start(w1["t"][:, 0:3, :], w1t_src[:, 0:3, :])
            nc.sync.dma_start(w1["t"][:, 3:6, :], w1t_src[:, 3:6, :])
            w2["t"] = wbig.tile([128, KT_H1, H2], bf16, tag="w2t", name="w2t")
            nc.sync.dma_start(
                w2["t"][:], d_w2t.ap().rearrange("p (kt n) -> p kt n", kt=KT_H1)
            )
            wb = [wtl[:, :, 0:H2], wtl[:, :, H2 : 2 * H2]]
            # per o: [:, kt, 2*H2 + 2*o] = head col, [:, kt, 2*H2 + 2*o + 1] = tail col

            # views into pkg
            cmat = pkg[:, 0 : 2 * H1].rearrange("p (s n) -> p s n", s=2)
            onehot = pkg[:, 2 * H1 : 2 * H1 + E]
            ones_t = pkg[0:1, 2 * H1 + E : 2 * H1 + 2 * E]

            b1 = {"h": smf[:, 0:MT_H1], "t": smf[:, MT_H1 : 2 * MT_H1]}
            b2 = {
                "h": smf[:, 2 * MT_H1 : 2 * MT_H1 + MT_H2],
                "t": smf[:, 2 * MT_H1 + MT_H2 : 2 * MT_H1 + 2 * MT_H2],
            }
            blin = smf[0:1, 2 * MT_H1 + 2 * MT_H2 : SF_COLS]

            # copy engines alternate to halve serial copy chains
            def copy(i, dst, src):
                if i % 2:
                    nc.scalar.activation(
                        dst, src, mybir.ActivationFunctionType.Identity
                    )
                else:
                    nc.vector.tensor_copy(dst, src)

            # ---- pooled^T -> entT  (kt-outer over hs chunks) ----
            entT = act.tile([128, KT_D, E], bf16, tag="entT")
            pool_ps = [
                ps.tile([128, E], f32, tag="mm", name=f"pp{m}") for m in range(MT_D)
            ]
            for kt in range(KT_T):
                for mt in range(MT_D):
                    nc.tensor.matmul(
                        pool_ps[mt][:],
                        hs[:, kt, mt * 128 : (mt + 1) * 128],
                        maskn[:, kt, :],
                        start=(kt == 0),
                        stop=(kt == KT_T - 1),
                    )
            for mt in range(MT_D):
                copy(mt, entT[:, mt, :], pool_ps[mt][:])

            # ---- ffnn chains on 128-entity subsets:
            #      head = cols 0:128, tail = cols 128:256 ----
            h2T = {}

            def ffnn(side):
                si = 0 if side == "h" else 1
                lo = si * EH
                h1T = act.tile(
                    [128, KT_H1, EH], bf16, tag=f"h1T{side}", name=f"h1T{side}"
                )
                accs = [
                    ps.tile([128, EH], f32, tag="mm", name=f"l1{side}{m}")
                    for m in range(MT_H1)
                ]
                for kt in range(KT_D):
                    for mt in range(MT_H1):
                        nc.tensor.matmul(
                            accs[mt][:],
                            w1[side][:, kt, mt * 128 : (mt + 1) * 128],
                            entT[:, kt, lo : lo + EH],
                            start=(kt == 0),
                            stop=False,
                        )
                # label-embedding contribution: K=5 one-hot matmul, C on host
                for mt in range(MT_H1):
                    nc.tensor.matmul(
                        accs[mt][:],
                        cmat[:, si, mt * 128 : (mt + 1) * 128],
                        onehot[:, lo : lo + EH],
                        start=False,
                        stop=True,
                    )
                for mt in range(MT_H1):
                    nc.scalar.activation(
                        h1T[:, mt, :],
                        accs[mt][:],
                        mybir.ActivationFunctionType.Relu,
                        bias=b1[side][:, mt : mt + 1],
                    )
                h2T[side] = act.tile(
                    [128, KT_H2, EH], bf16, tag=f"h2T{side}", name=f"h2T{side}"
                )
                accs2 = [
                    ps.tile([128, EH], f32, tag="mm", name=f"l2{side}{m}")
                    for m in range(MT_H2)
                ]
                for kt in range(KT_H1):
                    for mt in range(MT_H2):
                        nc.tensor.matmul(
                            accs2[mt][:],
                            w2[side][:, kt, mt * 128 : (mt + 1) * 128],
                            h1T[:, kt, :],
                            start=(kt == 0),
                            stop=(kt == KT_H1 - 1),
                        )
                for mt in range(MT_H2):
                    nc.scalar.activation(
                        h2T[side][:, mt, :],
                        accs2[mt][:],
                        mybir.ActivationFunctionType.Relu,
                        bias=b2[side][:, mt : mt + 1],
                    )

            ffnn("h")

            # ---- N_o^T [H2, EH] and linh [1, EH] for both o ----
            nT = []
            linh = []
            for o in range(OUT):
                nTo = act.tile([128, KT_H2, EH], bf16, tag=f"nT{o}", name=f"nT{o}")
                accs = [
                    ps.tile([128, EH], f32, tag="mm", name=f"nt{o}{m}")
                    for m in range(MT_H2)
                ]
                for kt in range(KT_H2):
                    for mt in range(MT_H2):
                        nc.tensor.matmul(
                            accs[mt][:],
                            wb[o][:, kt, mt * 128 : (mt + 1) * 128],
                            h2T["h"][:, kt, :],
                            start=(kt == 0),
                            stop=(kt == KT_H2 - 1),
                        )
                for mt in range(MT_H2):
                    copy(mt, nTo[:, mt, :], accs[mt][:])
                nT.append(nTo)

                lh = act.tile([1, EH], bf16, tag=f"linh{o}", name=f"linh{o}")
                p = ps1.tile([1, EH], f32, tag="lin")
                for kt in range(KT_H2):
                    nc.tensor.matmul(
                        p[:],
                        wtl[:, kt, 2 * H2 + 2 * o : 2 * H2 + 2 * o + 1],
                        h2T["h"][:, kt, :],
                        start=(kt == 0),
                        stop=(kt == KT_H2 - 1),
                    )
                nc.vector.tensor_copy(lh[:], p[:])
                linh.append(lh)

            ffnn("t")

            lint = []
            for o in range(OUT):
                lt = act.tile([1, EH], bf16, tag=f"lint{o}", name=f"lint{o}")
                p = ps1.tile([1, EH], f32, tag="lin")
                for kt in range(KT_H2):
                    nc.tensor.matmul(
                        p[:],
                        wtl[:, kt, 2 * H2 + 2 * o + 1 : 2 * H2 + 2 * o + 2],
                        h2T["t"][:, kt, :],
                        start=(kt == 0),
                        stop=(kt == KT_H2 - 1),
                    )
                # + b_lin[o] folded in via bias
                nc.scalar.activation(
                    lt[:],
                    p[:],
                    mybir.ActivationFunctionType.Identity,
                    bias=blin[:, o : o + 1],
                )
                lint.append(lt)

            # ---- table quadrant for this core: [128, OUT, EH] ----
            slab = act.tile([128, OUT, EH], bf16, tag="slab")
            slab_dst = d_slab.ap().rearrange("p (o n) -> p o n", o=OUT)
            for o in range(OUT):
                p = ps.tile([128, EH], f32, tag="mm")
                for kt in range(KT_H2):
                    nc.tensor.matmul(
                        p[:],
                        nT[o][:, kt, :],
                        h2T["t"][:, kt, :],
                        start=(kt == 0),
                        stop=False,
                    )
                nc.tensor.matmul(
                    p[:], linh[o][:], ones_t[:, 0:EH], start=False, stop=False
                )
                nc.tensor.matmul(
                    p[:], ones_t[:, 0:128], lint[o][:], start=False, stop=True
                )
                copy(o, slab[:, o, :], p[:])
                # ship each o-slab as soon as it's ready
                nc.sync.dma_start(slab_dst[:, o : o + 1, :], slab[:, o : o + 1, :])

    nc.compile()
    _cache[0] = nc
    return nc


def _pack(w, kt, dtype=ml_dtypes.bfloat16):
    """[kt*128, n] row-major -> [128, kt*n] partition-packed."""
    n = w.shape[1]
    return np.ascontiguousarray(
        w.reshape(kt, 128, n).transpose(1, 0, 2).reshape(128, kt * n).astype(dtype)
    )


def _prep_host(inputs):
    """Host-side input packing -> per-core in_maps + assembly info."""
    hs = np.asarray(inputs["hidden_states"], dtype=np.float32)
    start = np.asarray(inputs["entity_start"]).astype(np.int64)
    end = np.asarray(inputs["entity_end"]).astype(np.int64)
    label = np.asarray(inputs["entity_label"]).astype(np.int64)

    t = np.arange(T)
    mask = (
        (t[None, None, :] >= start[:, :, None]) & (t[None, None, :] < end[:, :, None])
    ).astype(np.float32)  # [B,E,T]
    counts = np.maximum(mask.sum(-1, keepdims=True), 1.0)
    masknT = (mask / counts).transpose(0, 2, 1)  # [B,T,E]

    def f32(x):
        return np.ascontiguousarray(np.asarray(x, dtype=np.float32))

    bf = ml_dtypes.bfloat16
    w_bil = f32(inputs["W_bil"])
    w_lin = f32(inputs["W_lin"])
    b_lin = f32(inputs["b_lin"])
    emb_all = f32(inputs["entity_emb_w"])

    # Wbil_0 | Wbil_1 | wlin cols interleaved per o as [head_o, tail_o]
    wb0 = _pack(w_bil[0], KT_H2, np.float32).reshape(128, KT_H2, H2)
    wb1 = _pack(w_bil[1], KT_H2, np.float32).reshape(128, KT_H2, H2)
    wl = np.stack(
        [
            w_lin[:H2, 0].reshape(KT_H2, 128).T,
            w_lin[H2:, 0].reshape(KT_H2, 128).T,
            w_lin[:H2, 1].reshape(KT_H2, 128).T,
            w_lin[H2:, 1].reshape(KT_H2, 128).T,
        ],
        axis=2,
    )  # [128, KT_H2, 4]
    region1 = np.concatenate([wb0, wb1], axis=2).reshape(128, -1)
    region2 = wl.reshape(128, -1)
    wtl = np.ascontiguousarray(
        np.concatenate([region1, region2], axis=1).astype(bf)
    )

    smf = np.zeros((128, SF_COLS), np.float32)
    smf[:, 0:MT_H1] = f32(inputs["bh1"]).reshape(MT_H1, 128).T
    smf[:, MT_H1 : 2 * MT_H1] = f32(inputs["bt1"]).reshape(MT_H1, 128).T
    smf[:, 2 * MT_H1 : 2 * MT_H1 + MT_H2] = f32(inputs["bh2"]).reshape(MT_H2, 128).T
    smf[:, 2 * MT_H1 + MT_H2 : 2 * MT_H1 + 2 * MT_H2] = (
        f32(inputs["bt2"]).reshape(MT_H2, 128).T
    )
    smf[0, 2 * MT_H1 + 2 * MT_H2 : SF_COLS] = b_lin

    # layer-1 split: W1a = W1[:D] on device; C = emb @ W1[D:] folded on host
    w1a = {}
    cfold = {}
    for s, key in (("h", "Wh1"), ("t", "Wt1")):
        w1f = f32(inputs[key])
        w1a[s] = _pack(w1f[:D], KT_D)
        cfold[s] = (emb_all @ w1f[D:]).astype(np.float32)  # [NL, H1]

    shared = {
        "W1h": w1a["h"],
        "W1t": w1a["t"],
        "W2h": _pack(f32(inputs["Wh2"]), KT_H1),
        "W2t": _pack(f32(inputs["Wt2"]), KT_H1),
        "Wtl": wtl,
        "smf": smf,
    }

    in_maps = []
    for i in range(N_CORES):
        b, q = divmod(i, 4)
        r, c = divmod(q, 2)  # row-half, col-half of the table quadrant
        sel = np.concatenate(
            [np.arange(EH * r, EH * r + EH), np.arange(EH * c, EH * c + EH)]
        )  # [head subset | tail subset]
        mrot = np.ascontiguousarray(masknT[b][:, sel])
        lab = label[b][sel]  # [E]
        onehot = (lab[None, :] == np.arange(NL)[:, None]).astype(np.float32)  # [NL,E]
        pkg = np.zeros((NL, PKG_COLS), np.float32)
        pkg[:, 0:H1] = cfold["h"]
        pkg[:, H1 : 2 * H1] = cfold["t"]
        pkg[:, 2 * H1 : 2 * H1 + E] = onehot
        pkg[0, 2 * H1 + E :] = 1.0  # ones row
        mm = dict(shared)
        mm["hs"] = _pack(hs[b], KT_T)
        mm["masknT"] = _pack(mrot, KT_T)
        mm["pkg"] = pkg.astype(bf)
        in_maps.append(mm)

    head_idx = np.asarray(inputs["head_idx"]).astype(np.int64)
    tail_idx = np.asarray(inputs["tail_idx"]).astype(np.int64)
    return in_maps, (head_idx, tail_idx), 0


def kernel(**inputs) -> np.ndarray:
    in_maps, (head_idx, tail_idx), ni = _prep_host(inputs)
    nc = _build(ni)
    res = run_bass_kernel_spmd(nc, in_maps, list(range(N_CORES)))
    out = np.zeros((B, P, OUT), np.float32)
    for b in range(B):
        slabs = np.stack(
            [
                res.results[4 * b + q]["slab"].reshape(128, OUT, EH).astype(np.float32)
                for q in range(4)
            ]
        )  # [q, 128, OUT, EH]; q = 2*r + c
        e1, e2 = head_idx[b], tail_idx[b]
        q = 2 * (e1 // EH) + (e2 // EH)
        out[b] = slabs[q, e1 % EH, :, e2 % EH]
    return out


# revision 21
# speedup vs baseline: 1.6549x; 1.1374x over previous
"""Trainium2 Bass kernel for nn_CellDecoder (span-pool + ffnn + biaffine pairs).

Strategy: head_idx/tail_idx only reference E=256 entities, so instead of
computing the biaffine per pair (P=65536), the cores build the full E x E
biaffine logit table (small matmuls). The per-pair work is a pure table
lookup with host-known indices, done during the host-side unshard/assembly
step, so the device kernel ships the dense table.

Sharding: 8 cores = batch (2) x e1-half (2) x e2-half (2). Each core
computes one 128x128 quadrant of the logit table (for both output logits):
the head ffnn chain runs on its 128 row-entities and the tail chain on its
128 column-entities. Per-core "which entities" is steered purely through
the inputs (SPMD program identical on all cores).

Key algebraic optimization vs the earlier version: the label-embedding
half of layer 1 is folded on the host:
    ent_repr @ W1 = pooled @ W1[:D] + emb[label] @ W1[D:]
and emb @ W1[D:] is a weights-only product -> precompute C = emb_w @ W1b
([5, H1] per side) on host; apply on device as a K=5 one-hot matmul that
rides the same PSUM accumulation as layer 1. This halves W1 DMA bytes
(2.36MB), removes the embedding DMA (0.46MB), and halves layer-1 tensor
cycles.

Perf notes:
- Everything is bf16; rel err ~5e-3, well under the 2e-2 gate.
- DMA issue is spread over four engine rings (scalar/sync/vector/gpsimd)
  in exact consumption order so the tensor engine starts pooling as soon
  as mask + first hs chunk land, and never starves afterwards.
- Matmul cost on TRN2 is ~(128 LDWEIGHTS + N moving) cycles; the kernel
  keeps N as large as the sharding allows.
- psum->sbuf copies alternate vector/scalar engines.
"""

import os

os.environ.setdefault("JAX_PLATFORMS", "axon,cpu")

import numpy as np
import ml_dtypes

import concourse.bass as bass
import concourse.tile as tile
from concourse import bacc, mybir
from concourse.bass_utils import run_bass_kernel_spmd

dt = mybir.dt

B, T, D, E, P = 2, 512, 768, 256, 65536
MLP = 2 * D  # 1536
H1, H2 = MLP // 2, MLP // 4  # 768, 384
NL = 5
OUT = 2
N_CORES = 8
EH = 128  # table rows/cols per core (quadrant edge)

KT_D = D // 128  # 6   (layer-1 contraction tiles over pooled part only)
KT_H1 = H1 // 128  # 6
KT_H2 = H2 // 128  # 3
KT_T = T // 128  # 4
MT_D = D // 128  # 6
MT_H1 = H1 // 128  # 6
MT_H2 = H2 // 128  # 3

SF_COLS = 2 * MT_H1 + 2 * MT_H2 + OUT  # b1h, b1t, b2h, b2t, blin = 20
PKG_COLS = 2 * H1 + E + E  # C_h | C_t | onehot | ones-block

_cache: dict = {}


def _build(ni: int = 0):
    """Build + compile the SPMD program (ni unused, kept for test.py interface)."""
    if 0 in _cache:
        return _cache[0]

    nc = bacc.Bacc("TRN2", target_bir_lowering=False, debug=False, num_devices=N_CORES)

    f32 = dt.float32
    bf16 = dt.bfloat16

    # [128, cols] host-packed operand tensors
    d_hs = nc.dram_tensor("hs", [128, KT_T * D], bf16, kind="ExternalInput")
    d_maskn = nc.dram_tensor("masknT", [128, KT_T * E], bf16, kind="ExternalInput")
    d_w1h = nc.dram_tensor("W1h", [128, KT_D * H1], bf16, kind="ExternalInput")
    d_w1t = nc.dram_tensor("W1t", [128, KT_D * H1], bf16, kind="ExternalInput")
    d_w2h = nc.dram_tensor("W2h", [128, KT_H1 * H2], bf16, kind="ExternalInput")
    d_w2t = nc.dram_tensor("W2t", [128, KT_H1 * H2], bf16, kind="ExternalInput")
    # Wbil_0 | Wbil_1 | wlin (head/tail cols for both o)
    d_wtl = nc.dram_tensor(
        "Wtl", [128, 2 * KT_H2 * H2 + 2 * KT_H2 * OUT], bf16, kind="ExternalInput"
    )
    # [5, .]: C_h | C_t | one-hot labels | ones-block (row 0 = 1.0)
    d_pkg = nc.dram_tensor("pkg", [NL, PKG_COLS], bf16, kind="ExternalInput")
    d_smf = nc.dram_tensor("smf", [128, SF_COLS], f32, kind="ExternalInput")
    # output: this core's table quadrant per o, columns in subset order
    d_slab = nc.dram_tensor("slab", [128, OUT * EH], bf16, kind="ExternalOutput")

    with tile.TileContext(nc) as tc:
        with (
            tc.tile_pool(name="wbig", bufs=1) as wbig,
            tc.tile_pool(name="wsml", bufs=1) as wsml,
            tc.tile_pool(name="act", bufs=1) as act,
            tc.tile_pool(name="ps", bufs=6, space="PSUM") as ps,
            tc.tile_pool(name="ps1", bufs=2, space="PSUM") as ps1,
        ):
            # ---- DMA issue in consumption order ----
            # (No PE warm-up: TRN2's power governor throttles the tensor
            # engine to 50% util in proportion to activity — junk matmuls
            # increase throttle time and net-lose.)
            # scalar ring: maskn (2 chunks so pooling kt=0 starts early),
            # then the small stuff
            maskn = wsml.tile([128, KT_T, E], bf16, tag="maskn", name="maskn")
            maskn_src = d_maskn.ap().rearrange("p (kt n) -> p kt n", kt=KT_T)
            nc.scalar.dma_start(maskn[:, 0:2, :], maskn_src[:, 0:2, :])
            nc.scalar.dma_start(maskn[:, 2:4, :], maskn_src[:, 2:4, :])
            smf = wsml.tile([128, SF_COLS], f32, tag="smf", name="smf")
            nc.scalar.dma_start(smf[:], d_smf.ap())
            pkg = wsml.tile([NL, PKG_COLS], bf16, tag="pkg", name="pkg")
            nc.scalar.dma_start(pkg[:], d_pkg.ap())

            # sync ring: ALL bulk tensors, in exact consumption order.
            # (Do NOT spread bulk across rings: DMA engines round-robin
            # across queued descriptors, so multiple bulk rings destroy the
            # arrival ordering and starve the tensor engine mid-kernel.)
            hs = wbig.tile([128, KT_T, D], bf16, tag="hs", name="hs")
            hs_src = d_hs.ap().rearrange("p (kt n) -> p kt n", kt=KT_T)
            for kt in range(KT_T):
                nc.sync.dma_start(hs[:, kt : kt + 1, :], hs_src[:, kt : kt + 1, :])

            w1 = {}
            w2 = {}
            w1["h"] = wbig.tile([128, KT_D, H1], bf16, tag="w1h", name="w1h")
            w1h_src = d_w1h.ap().rearrange("p (kt n) -> p kt n", kt=KT_D)
            nc.sync.dma_start(w1["h"][:, 0:3, :], w1h_src[:, 0:3, :])
            nc.sync.dma_start(w1["h"][:, 3:6, :], w1h_src[:, 3:6, :])
            w2["h"] = wbig.tile([128, KT_H1, H2], bf16, tag="w2h", name="w2h")
            nc.sync.dma_start(
                w2["h"][:], d_w2h.ap().rearrange("p (kt n) -> p kt n", kt=KT_H1)
            )

            # wtl ships BEFORE w1t/w2t: the tensor queue is in-order and the
            # nT matmuls (which need wtl) sit between ffnn-h and ffnn-t in
            # program order — wtl arriving last would stall the whole queue.
            wtl = wsml.tile([128, KT_H2, 2 * H2 + 2 * OUT], bf16, tag="wtl", name="wtl")
            nc.sync.dma_start(
                wtl[:, :, 0 : 2 * H2],
                d_wtl.ap()[:, 0 : 2 * KT_H2 * H2].rearrange(
                    "p (kt n) -> p kt n", kt=KT_H2
                ),
            )
            nc.sync.dma_start(
                wtl[:, :, 2 * H2 :],
                d_wtl.ap()[:, 2 * KT_H2 * H2 :].rearrange(
                    "p (kt n) -> p kt n", kt=KT_H2
                ),
            )
            w1["t"] = wbig.tile([128, KT_D, H1], bf16, tag="w1t", name="w1t")
            w1t_src = d_w1t.ap().rearrange("p (kt n) -> p kt n", kt=KT_D)
            nc.sync.dma_start(w1["t"][:, 0:3, :], w1t_src[:, 0:3, :])
            nc.sync.dma_start(w1["t"][:, 3:6, :], w1t_src[:, 3:6, :])
            w2["t"] = wbig.tile([128, KT_H1, H2], bf16, tag="w2t", name="w2t")
            nc.sync.dma_start(
                w2["t"][:], d_w2t.ap().rearrange("p (kt n) -> p kt n", kt=KT_H1)
            )
            wb = [wtl[:, :, 0:H2], wtl[:, :, H2 : 2 * H2]]
            # per o: [:, kt, 2*H2 + 2*o] = head col, [:, kt, 2*H2 + 2*o + 1] = tail col

            # views into pkg
            cmat = pkg[:, 0 : 2 * H1].rearrange("p (s n) -> p s n", s=2)
            onehot = pkg[:, 2 * H1 : 2 * H1 + E]
            ones_t = pkg[0:1, 2 * H1 + E : 2 * H1 + 2 * E]

            b1 = {"h": smf[:, 0:MT_H1], "t": smf[:, MT_H1 : 2 * MT_H1]}
            b2 = {
                "h": smf[:, 2 * MT_H1 : 2 * MT_H1 + MT_H2],
                "t": smf[:, 2 * MT_H1 + MT_H2 : 2 * MT_H1 + 2 * MT_H2],
            }
            blin = smf[0:1, 2 * MT_H1 + 2 * MT_H2 : SF_COLS]

            # copy engines alternate to halve serial copy chains
            def copy(i, dst, src):
                if i % 2:
                    nc.scalar.activation(
                        dst, src, mybir.ActivationFunctionType.Identity
                    )
                else:
                    nc.vector.tensor_copy(dst, src)

            # relu+bias, alternating scalar/vector engines
            def relu_bias(i, dst, src, bias):
                if i % 2:
                    nc.vector.tensor_scalar(
                        dst,
                        src,
                        bias,
                        0.0,
                        mybir.AluOpType.add,
                        mybir.AluOpType.max,
                    )
                else:
                    nc.scalar.activation(
                        dst, src, mybir.ActivationFunctionType.Relu, bias=bias
                    )

            # ---- pooled^T -> entT  (kt-outer over hs chunks) ----
            entT = act.tile([128, KT_D, E], bf16, tag="entT")
            pool_ps = [
                ps.tile([128, E], f32, tag="mm", name=f"pp{m}") for m in range(MT_D)
            ]
            for kt in range(KT_T):
                for mt in range(MT_D):
                    nc.tensor.matmul(
                        pool_ps[mt][:],
                        hs[:, kt, mt * 128 : (mt + 1) * 128],
                        maskn[:, kt, :],
                        start=(kt == 0),
                        stop=(kt == KT_T - 1),
                    )
            for mt in range(MT_D):
                copy(mt, entT[:, mt, :], pool_ps[mt][:])

            # ---- ffnn chains on 128-entity subsets:
            #      head = cols 0:128, tail = cols 128:256 ----
            h2T = {}

            def ffnn(side):
                si = 0 if side == "h" else 1
                lo = si * EH
                h1T = act.tile(
                    [128, KT_H1, EH], bf16, tag=f"h1T{side}", name=f"h1T{side}"
                )
                accs = [
                    ps.tile([128, EH], f32, tag="mm", name=f"l1{side}{m}")
                    for m in range(MT_H1)
                ]
                for kt in range(KT_D):
                    for mt in range(MT_H1):
                        nc.tensor.matmul(
                            accs[mt][:],
                            w1[side][:, kt, mt * 128 : (mt + 1) * 128],
                            entT[:, kt, lo : lo + EH],
                            start=(kt == 0),
                            stop=False,
                        )
                # label-embedding contribution: K=5 one-hot matmul, C on host
                for mt in range(MT_H1):
                    nc.tensor.matmul(
                        accs[mt][:],
                        cmat[:, si, mt * 128 : (mt + 1) * 128],
                        onehot[:, lo : lo + EH],
                        start=False,
                        stop=True,
                    )
                for mt in range(MT_H1):
                    relu_bias(
                        mt, h1T[:, mt, :], accs[mt][:], b1[side][:, mt : mt + 1]
                    )
                h2T[side] = act.tile(
                    [128, KT_H2, EH], bf16, tag=f"h2T{side}", name=f"h2T{side}"
                )
                accs2 = [
                    ps.tile([128, EH], f32, tag="mm", name=f"l2{side}{m}")
                    for m in range(MT_H2)
                ]
                for kt in range(KT_H1):
                    for mt in range(MT_H2):
                        nc.tensor.matmul(
                            accs2[mt][:],
                            w2[side][:, kt, mt * 128 : (mt + 1) * 128],
                            h1T[:, kt, :],
                            start=(kt == 0),
                            stop=(kt == KT_H1 - 1),
                        )
                for mt in range(MT_H2):
                    relu_bias(
                        mt,
                        h2T[side][:, mt, :],
                        accs2[mt][:],
                        b2[side][:, mt : mt + 1],
                    )

            ffnn("h")

            # ---- N_o^T [H2, EH] and linh [1, EH] for both o ----
            nT = []
            linh = []
            for o in range(OUT):
                nTo = act.tile([128, KT_H2, EH], bf16, tag=f"nT{o}", name=f"nT{o}")
                accs = [
                    ps.tile([128, EH], f32, tag="mm", name=f"nt{o}{m}")
                    for m in range(MT_H2)
                ]
                for kt in range(KT_H2):
                    for mt in range(MT_H2):
                        nc.tensor.matmul(
                            accs[mt][:],
                            wb[o][:, kt, mt * 128 : (mt + 1) * 128],
                            h2T["h"][:, kt, :],
                            start=(kt == 0),
                            stop=(kt == KT_H2 - 1),
                        )
                for mt in range(MT_H2):
                    copy(mt, nTo[:, mt, :], accs[mt][:])
                nT.append(nTo)

                lh = act.tile([1, EH], bf16, tag=f"linh{o}", name=f"linh{o}")
                p = ps1.tile([1, EH], f32, tag="lin")
                for kt in range(KT_H2):
                    nc.tensor.matmul(
                        p[:],
                        wtl[:, kt, 2 * H2 + 2 * o : 2 * H2 + 2 * o + 1],
                        h2T["h"][:, kt, :],
                        start=(kt == 0),
                        stop=(kt == KT_H2 - 1),
                    )
                nc.vector.tensor_copy(lh[:], p[:])
                linh.append(lh)

            ffnn("t")

            lint = []
            for o in range(OUT):
                lt = act.tile([1, EH], bf16, tag=f"lint{o}", name=f"lint{o}")
                p = ps1.tile([1, EH], f32, tag="lin")
                for kt in range(KT_H2):
                    nc.tensor.matmul(
                        p[:],
                        wtl[:, kt, 2 * H2 + 2 * o + 1 : 2 * H2 + 2 * o + 2],
                        h2T["t"][:, kt, :],
                        start=(kt == 0),
                        stop=(kt == KT_H2 - 1),
                    )
                # + b_lin[o] folded in via bias
                nc.scalar.activation(
                    lt[:],
                    p[:],
                    mybir.ActivationFunctionType.Identity,
                    bias=blin[:, o : o + 1],
                )
                lint.append(lt)

            # ---- table quadrant for this core: [128, OUT, EH] ----
            slab = act.tile([128, OUT, EH], bf16, tag="slab")
            slab_dst = d_slab.ap().rearrange("p (o n) -> p o n", o=OUT)
            for o in range(OUT):
                p = ps.tile([128, EH], f32, tag="mm")
                for kt in range(KT_H2):
                    nc.tensor.matmul(
                        p[:],
                        nT[o][:, kt, :],
                        h2T["t"][:, kt, :],
                        start=(kt == 0),
                        stop=False,
                    )
                nc.tensor.matmul(
                    p[:], linh[o][:], ones_t[:, 0:EH], start=False, stop=False
                )
                nc.tensor.matmul(
                    p[:], ones_t[:, 0:128], lint[o][:], start=False, stop=True
                )
                copy(o, slab[:, o, :], p[:])
                # ship each o-slab as soon as it's ready
                nc.sync.dma_start(slab_dst[:, o : o + 1, :], slab[:, o : o + 1, :])

    nc.compile()
    _cache[0] = nc
    return nc


def _pack(w, kt, dtype=ml_dtypes.bfloat16):
    """[kt*128, n] row-major -> [128, kt*n] partition-packed."""
    n = w.shape[1]
    return np.ascontiguousarray(
        w.reshape(kt, 128, n).transpose(1, 0, 2).reshape(128, kt * n).astype(dtype)
    )


def _prep_host(inputs):
    """Host-side input packing -> per-core in_maps + assembly info."""
    hs = np.asarray(inputs["hidden_states"], dtype=np.float32)
    start = np.asarray(inputs["entity_start"]).astype(np.int64)
    end = np.asarray(inputs["entity_end"]).astype(np.int64)
    label = np.asarray(inputs["entity_label"]).astype(np.int64)

    t = np.arange(T)
    mask = (
        (t[None, None, :] >= start[:, :, None]) & (t[None, None, :] < end[:, :, None])
    ).astype(np.float32)  # [B,E,T]
    counts = np.maximum(mask.sum(-1, keepdims=True), 1.0)
    masknT = (mask / counts).transpose(0, 2, 1)  # [B,T,E]

    def f32(x):
        return np.ascontiguousarray(np.asarray(x, dtype=np.float32))

    bf = ml_dtypes.bfloat16
    w_bil = f32(inputs["W_bil"])
    w_lin = f32(inputs["W_lin"])
    b_lin = f32(inputs["b_lin"])
    emb_all = f32(inputs["entity_emb_w"])

    # Wbil_0 | Wbil_1 | wlin cols interleaved per o as [head_o, tail_o]
    wb0 = _pack(w_bil[0], KT_H2, np.float32).reshape(128, KT_H2, H2)
    wb1 = _pack(w_bil[1], KT_H2, np.float32).reshape(128, KT_H2, H2)
    wl = np.stack(
        [
            w_lin[:H2, 0].reshape(KT_H2, 128).T,
            w_lin[H2:, 0].reshape(KT_H2, 128).T,
            w_lin[:H2, 1].reshape(KT_H2, 128).T,
            w_lin[H2:, 1].reshape(KT_H2, 128).T,
        ],
        axis=2,
    )  # [128, KT_H2, 4]
    region1 = np.concatenate([wb0, wb1], axis=2).reshape(128, -1)
    region2 = wl.reshape(128, -1)
    wtl = np.ascontiguousarray(
        np.concatenate([region1, region2], axis=1).astype(bf)
    )

    smf = np.zeros((128, SF_COLS), np.float32)
    smf[:, 0:MT_H1] = f32(inputs["bh1"]).reshape(MT_H1, 128).T
    smf[:, MT_H1 : 2 * MT_H1] = f32(inputs["bt1"]).reshape(MT_H1, 128).T
    smf[:, 2 * MT_H1 : 2 * MT_H1 + MT_H2] = f32(inputs["bh2"]).reshape(MT_H2, 128).T
    smf[:, 2 * MT_H1 + MT_H2 : 2 * MT_H1 + 2 * MT_H2] = (
        f32(inputs["bt2"]).reshape(MT_H2, 128).T
    )
    smf[0, 2 * MT_H1 + 2 * MT_H2 : SF_COLS] = b_lin

    # layer-1 split: W1a = W1[:D] on device; C = emb @ W1[D:] folded on host
    w1a = {}
    cfold = {}
    for s, key in (("h", "Wh1"), ("t", "Wt1")):
        w1f = f32(inputs[key])
        w1a[s] = _pack(w1f[:D], KT_D)
        cfold[s] = (emb_all @ w1f[D:]).astype(np.float32)  # [NL, H1]

    shared = {
        "W1h": w1a["h"],
        "W1t": w1a["t"],
        "W2h": _pack(f32(inputs["Wh2"]), KT_H1),
        "W2t": _pack(f32(inputs["Wt2"]), KT_H1),
        "Wtl": wtl,
        "smf": smf,
    }

    in_maps = []
    for i in range(N_CORES):
        b, q = divmod(i, 4)
        r, c = divmod(q, 2)  # row-half, col-half of the table quadrant
        sel = np.concatenate(
            [np.arange(EH * r, EH * r + EH), np.arange(EH * c, EH * c + EH)]
        )  # [head subset | tail subset]
        mrot = np.ascontiguousarray(masknT[b][:, sel])
        lab = label[b][sel]  # [E]
        onehot = (lab[None, :] == np.arange(NL)[:, None]).astype(np.float32)  # [NL,E]
        pkg = np.zeros((NL, PKG_COLS), np.float32)
        pkg[:, 0:H1] = cfold["h"]
        pkg[:, H1 : 2 * H1] = cfold["t"]
        pkg[:, 2 * H1 : 2 * H1 + E] = onehot
        pkg[0, 2 * H1 + E :] = 1.0  # ones row
        mm = dict(shared)
        mm["hs"] = _pack(hs[b], KT_T)
        mm["masknT"] = _pack(mrot, KT_T)
        mm["pkg"] = pkg.astype(bf)
        in_maps.append(mm)

    head_idx = np.asarray(inputs["head_idx"]).astype(np.int64)
    tail_idx = np.asarray(inputs["tail_idx"]).astype(np.int64)
    return in_maps, (head_idx, tail_idx), 0


def kernel(**inputs) -> np.ndarray:
    in_maps, (head_idx, tail_idx), ni = _prep_host(inputs)
    nc = _build(ni)
    res = run_bass_kernel_spmd(nc, in_maps, list(range(N_CORES)))
    out = np.zeros((B, P, OUT), np.float32)
    for b in range(B):
        slabs = np.stack(
            [
                res.results[4 * b + q]["slab"].reshape(128, OUT, EH).astype(np.float32)
                for q in range(4)
            ]
        )  # [q, 128, OUT, EH]; q = 2*r + c
        e1, e2 = head_idx[b], tail_idx[b]
        q = 2 * (e1 // EH) + (e2 // EH)
        out[b] = slabs[q, e1 % EH, :, e2 % EH]
    return out


# revision 29
# speedup vs baseline: 1.6677x; 1.0077x over previous
"""Trainium2 Bass kernel for nn_CellDecoder (span-pool + ffnn + biaffine pairs).

Strategy: head_idx/tail_idx only reference E=256 entities, so instead of
computing the biaffine per pair (P=65536), the cores build the full E x E
biaffine logit table (small matmuls). The per-pair work is a pure table
lookup with host-known indices, done during the host-side unshard/assembly
step, so the device kernel ships the dense table.

Sharding: 8 cores = batch (2) x e1-half (2) x e2-half (2). Each core
computes one 128x128 quadrant of the logit table (for both output logits):
the head ffnn chain runs on its 128 row-entities and the tail chain on its
128 column-entities. Per-core "which entities" is steered purely through
the inputs (SPMD program identical on all cores).

Key algebraic optimization vs the earlier version: the label-embedding
half of layer 1 is folded on the host:
    ent_repr @ W1 = pooled @ W1[:D] + emb[label] @ W1[D:]
and emb @ W1[D:] is a weights-only product -> precompute C = emb_w @ W1b
([5, H1] per side) on host; apply on device as a K=5 one-hot matmul that
rides the same PSUM accumulation as layer 1. This halves W1 DMA bytes
(2.36MB), removes the embedding DMA (0.46MB), and halves layer-1 tensor
cycles.

Perf notes:
- Everything is bf16; rel err ~5e-3, well under the 2e-2 gate.
- DMA issue is spread over four engine rings (scalar/sync/vector/gpsimd)
  in exact consumption order so the tensor engine starts pooling as soon
  as mask + first hs chunk land, and never starves afterwards.
- Matmul cost on TRN2 is ~(128 LDWEIGHTS + N moving) cycles; the kernel
  keeps N as large as the sharding allows.
- psum->sbuf copies alternate vector/scalar engines.
"""

import os

os.environ.setdefault("JAX_PLATFORMS", "axon,cpu")

import numpy as np
import ml_dtypes

import concourse.bass as bass
import concourse.tile as tile
from concourse import bacc, mybir
from concourse.bass_utils import run_bass_kernel_spmd

dt = mybir.dt

B, T, D, E, P = 2, 512, 768, 256, 65536
MLP = 2 * D  # 1536
H1, H2 = MLP // 2, MLP // 4  # 768, 384
NL = 5
OUT = 2
N_CORES = 8
EH = 128  # table rows/cols per core (quadrant edge)

KT_D = D // 128  # 6   (layer-1 contraction tiles over pooled part only)
KT_H1 = H1 // 128  # 6
KT_H2 = H2 // 128  # 3
KT_T = T // 128  # 4
MT_D = D // 128  # 6
MT_H1 = H1 // 128  # 6
MT_H2 = H2 // 128  # 3

SF_COLS = 2 * MT_H1 + 2 * MT_H2 + OUT  # b1h, b1t, b2h, b2t, blin = 20
PKG_COLS = 2 * H1 + E + E  # C_h | C_t | onehot | ones-block

_cache: dict = {}


def _build(ni: int = 0):
    """Build + compile the SPMD program (ni unused, kept for test.py interface)."""
    if 0 in _cache:
        return _cache[0]

    nc = bacc.Bacc("TRN2", target_bir_lowering=False, debug=False, num_devices=N_CORES)

    f32 = dt.float32
    bf16 = dt.bfloat16

    # [128, cols] host-packed operand tensors
    d_hs = nc.dram_tensor("hs", [128, KT_T * D], bf16, kind="ExternalInput")
    d_maskn = nc.dram_tensor("masknT", [128, KT_T * E], bf16, kind="ExternalInput")
    d_w1h = nc.dram_tensor("W1h", [128, KT_D * H1], bf16, kind="ExternalInput")
    d_w1t = nc.dram_tensor("W1t", [128, KT_D * H1], bf16, kind="ExternalInput")
    d_w2h = nc.dram_tensor("W2h", [128, KT_H1 * H2], bf16, kind="ExternalInput")
    d_w2t = nc.dram_tensor("W2t", [128, KT_H1 * H2], bf16, kind="ExternalInput")
    # Wbil_0 | Wbil_1 | wlin (head/tail cols for both o)
    d_wtl = nc.dram_tensor(
        "Wtl", [128, 2 * KT_H2 * H2 + 2 * KT_H2 * OUT], bf16, kind="ExternalInput"
    )
    # [5, .]: C_h | C_t | one-hot labels | ones-block (row 0 = 1.0)
    d_pkg = nc.dram_tensor("pkg", [NL, PKG_COLS], bf16, kind="ExternalInput")
    d_smf = nc.dram_tensor("smf", [128, SF_COLS], f32, kind="ExternalInput")
    # output: this core's table quadrant per o, columns in subset order
    d_slab = nc.dram_tensor("slab", [128, OUT * EH], bf16, kind="ExternalOutput")

    with tile.TileContext(nc) as tc:
        with (
            tc.tile_pool(name="wbig", bufs=1) as wbig,
            tc.tile_pool(name="wsml", bufs=1) as wsml,
            tc.tile_pool(name="act", bufs=1) as act,
            tc.tile_pool(name="ps", bufs=6, space="PSUM") as ps,
            tc.tile_pool(name="ps1", bufs=2, space="PSUM") as ps1,
        ):
            # ---- DMA issue in consumption order ----
            # (No PE warm-up: TRN2's power governor throttles the tensor
            # engine to 50% util in proportion to activity — junk matmuls
            # increase throttle time and net-lose.)
            # scalar ring: maskn (2 chunks so pooling kt=0 starts early),
            # then the small stuff
            maskn = wsml.tile([128, KT_T, E], bf16, tag="maskn", name="maskn")
            maskn_src = d_maskn.ap().rearrange("p (kt n) -> p kt n", kt=KT_T)
            nc.scalar.dma_start(maskn[:, 0:1, :], maskn_src[:, 0:1, :])
            nc.scalar.dma_start(maskn[:, 1:4, :], maskn_src[:, 1:4, :])
            smf = wsml.tile([128, SF_COLS], f32, tag="smf", name="smf")
            nc.scalar.dma_start(smf[:], d_smf.ap())
            pkg = wsml.tile([NL, PKG_COLS], bf16, tag="pkg", name="pkg")
            nc.scalar.dma_start(pkg[:], d_pkg.ap())

            # sync ring: ALL bulk tensors, in exact consumption order.
            # (Do NOT spread bulk across rings: DMA engines round-robin
            # across queued descriptors, so multiple bulk rings destroy the
            # arrival ordering and starve the tensor engine mid-kernel.)
            hs = wbig.tile([128, KT_T, D], bf16, tag="hs", name="hs")
            hs_src = d_hs.ap().rearrange("p (kt n) -> p kt n", kt=KT_T)
            for kt in range(KT_T):
                nc.sync.dma_start(hs[:, kt : kt + 1, :], hs_src[:, kt : kt + 1, :])

            w1 = {}
            w2 = {}
            w1["h"] = wbig.tile([128, KT_D, H1], bf16, tag="w1h", name="w1h")
            w1h_src = d_w1h.ap().rearrange("p (kt n) -> p kt n", kt=KT_D)
            for c in range(0, KT_D, 2):
                nc.sync.dma_start(w1["h"][:, c : c + 2, :], w1h_src[:, c : c + 2, :])
            w2["h"] = wbig.tile([128, KT_H1, H2], bf16, tag="w2h", name="w2h")
            nc.sync.dma_start(
                w2["h"][:], d_w2h.ap().rearrange("p (kt n) -> p kt n", kt=KT_H1)
            )

            # wtl ships BEFORE w1t/w2t: the tensor queue is in-order and the
            # nT matmuls (which need wtl) sit between ffnn-h and ffnn-t in
            # program order — wtl arriving last would stall the whole queue.
            # (host packs it kt-major as [128, KT_H2, 2*H2+2*OUT]: one DMA)
            wtl = wsml.tile([128, KT_H2, 2 * H2 + 2 * OUT], bf16, tag="wtl", name="wtl")
            nc.sync.dma_start(
                wtl[:],
                d_wtl.ap().rearrange("p (kt n) -> p kt n", kt=KT_H2),
            )
            w1["t"] = wbig.tile([128, KT_D, H1], bf16, tag="w1t", name="w1t")
            w1t_src = d_w1t.ap().rearrange("p (kt n) -> p kt n", kt=KT_D)
            for c in range(0, KT_D, 2):
                nc.sync.dma_start(w1["t"][:, c : c + 2, :], w1t_src[:, c : c + 2, :])
            w2["t"] = wbig.tile([128, KT_H1, H2], bf16, tag="w2t", name="w2t")
            nc.sync.dma_start(
                w2["t"][:], d_w2t.ap().rearrange("p (kt n) -> p kt n", kt=KT_H1)
            )
            wb = [wtl[:, :, 0:H2], wtl[:, :, H2 : 2 * H2]]
            # per o: [:, kt, 2*H2 + 2*o] = head col, [:, kt, 2*H2 + 2*o + 1] = tail
            # col

            # views into pkg
            cmat = pkg[:, 0 : 2 * H1].rearrange("p (s n) -> p s n", s=2)
            onehot = pkg[:, 2 * H1 : 2 * H1 + E]
            ones_t = pkg[0:1, 2 * H1 + E : 2 * H1 + 2 * E]

            b1 = {"h": smf[:, 0:MT_H1], "t": smf[:, MT_H1 : 2 * MT_H1]}
            b2 = {
                "h": smf[:, 2 * MT_H1 : 2 * MT_H1 + MT_H2],
                "t": smf[:, 2 * MT_H1 + MT_H2 : 2 * MT_H1 + 2 * MT_H2],
            }
            blin = smf[0:1, 2 * MT_H1 + 2 * MT_H2 : SF_COLS]

            # copy engines alternate to halve serial copy chains
            def copy(i, dst, src):
                if i % 2:
                    nc.scalar.activation(
                        dst, src, mybir.ActivationFunctionType.Identity
                    )
                else:
                    nc.vector.tensor_copy(dst, src)

            # relu+bias, alternating scalar/vector engines
            def relu_bias(i, dst, src, bias):
                if i % 2:
                    nc.vector.tensor_scalar(
                        dst,
                        src,
                        bias,
                        0.0,
                        mybir.AluOpType.add,
                        mybir.AluOpType.max,
                    )
                else:
                    nc.scalar.activation(
                        dst, src, mybir.ActivationFunctionType.Relu, bias=bias
                    )

            # ---- pooled^T -> entT  (kt-outer over hs chunks) ----
            entT = act.tile([128, KT_D, E], bf16, tag="entT")
            pool_ps = [
                ps.tile([128, E], f32, tag="mm", name=f"pp{m}") for m in range(MT_D)
            ]
            for kt in range(KT_T):
                for mt in range(MT_D):
                    nc.tensor.matmul(
                        pool_ps[mt][:],
                        hs[:, kt, mt * 128 : (mt + 1) * 128],
                        maskn[:, kt, :],
                        start=(kt == 0),
                        stop=(kt == KT_T - 1),
                    )
            for mt in range(MT_D):
                copy(mt, entT[:, mt, :], pool_ps[mt][:])

            # ---- ffnn chains on 128-entity subsets:
            #      head = cols 0:128, tail = cols 128:256 ----
            h2T = {}

            def ffnn(side):
                si = 0 if side == "h" else 1
                lo = si * EH
                h1T = act.tile(
                    [128, KT_H1, EH], bf16, tag=f"h1T{side}", name=f"h1T{side}"
                )
                accs = [
                    ps.tile([128, EH], f32, tag="mm", name=f"l1{side}{m}")
                    for m in range(MT_H1)
                ]
                for kt in range(KT_D):
                    for mt in range(MT_H1):
                        nc.tensor.matmul(
                            accs[mt][:],
                            w1[side][:, kt, mt * 128 : (mt + 1) * 128],
                            entT[:, kt, lo : lo + EH],
                            start=(kt == 0),
                            stop=False,
                        )
                # label-embedding contribution: K=5 one-hot matmul, C on host
                for mt in range(MT_H1):
                    nc.tensor.matmul(
                        accs[mt][:],
                        cmat[:, si, mt * 128 : (mt + 1) * 128],
                        onehot[:, lo : lo + EH],
                        start=False,
                        stop=True,
                    )
                for mt in range(MT_H1):
                    relu_bias(
                        mt, h1T[:, mt, :], accs[mt][:], b1[side][:, mt : mt + 1]
                    )
                h2T[side] = act.tile(
                    [128, KT_H2, EH], bf16, tag=f"h2T{side}", name=f"h2T{side}"
                )
                accs2 = [
                    ps.tile([128, EH], f32, tag="mm", name=f"l2{side}{m}")
                    for m in range(MT_H2)
                ]
                for kt in range(KT_H1):
                    for mt in range(MT_H2):
                        nc.tensor.matmul(
                            accs2[mt][:],
                            w2[side][:, kt, mt * 128 : (mt + 1) * 128],
                            h1T[:, kt, :],
                            start=(kt == 0),
                            stop=(kt == KT_H1 - 1),
                        )
                for mt in range(MT_H2):
                    relu_bias(
                        mt,
                        h2T[side][:, mt, :],
                        accs2[mt][:],
                        b2[side][:, mt : mt + 1],
                    )

            ffnn("h")

            # ---- N_o^T [H2, EH] and linh [1, EH] for both o ----
            nT = []
            linh = []
            for o in range(OUT):
                nTo = act.tile([128, KT_H2, EH], bf16, tag=f"nT{o}", name=f"nT{o}")
                accs = [
                    ps.tile([128, EH], f32, tag="mm", name=f"nt{o}{m}")
                    for m in range(MT_H2)
                ]
                for kt in range(KT_H2):
                    for mt in range(MT_H2):
                        nc.tensor.matmul(
                            accs[mt][:],
                            wb[o][:, kt, mt * 128 : (mt + 1) * 128],
                            h2T["h"][:, kt, :],
                            start=(kt == 0),
                            stop=(kt == KT_H2 - 1),
                        )
                for mt in range(MT_H2):
                    copy(mt, nTo[:, mt, :], accs[mt][:])
                nT.append(nTo)

                lh = act.tile([1, EH], bf16, tag=f"linh{o}", name=f"linh{o}")
                p = ps1.tile([1, EH], f32, tag="lin")
                for kt in range(KT_H2):
                    nc.tensor.matmul(
                        p[:],
                        wtl[:, kt, 2 * H2 + 2 * o : 2 * H2 + 2 * o + 1],
                        h2T["h"][:, kt, :],
                        start=(kt == 0),
                        stop=(kt == KT_H2 - 1),
                    )
                nc.vector.tensor_copy(lh[:], p[:])
                linh.append(lh)

            ffnn("t")

            lint = []
            for o in range(OUT):
                lt = act.tile([1, EH], bf16, tag=f"lint{o}", name=f"lint{o}")
                p = ps1.tile([1, EH], f32, tag="lin")
                for kt in range(KT_H2):
                    nc.tensor.matmul(
                        p[:],
                        wtl[:, kt, 2 * H2 + 2 * o + 1 : 2 * H2 + 2 * o + 2],
                        h2T["t"][:, kt, :],
                        start=(kt == 0),
                        stop=(kt == KT_H2 - 1),
                    )
                # + b_lin[o] folded in via bias
                nc.scalar.activation(
                    lt[:],
                    p[:],
                    mybir.ActivationFunctionType.Identity,
                    bias=blin[:, o : o + 1],
                )
                lint.append(lt)

            # ---- table quadrant for this core: [128, OUT, EH] ----
            slab = act.tile([128, OUT, EH], bf16, tag="slab")
            slab_dst = d_slab.ap().rearrange("p (o n) -> p o n", o=OUT)
            for o in range(OUT):
                p = ps.tile([128, EH], f32, tag="mm")
                for kt in range(KT_H2):
                    nc.tensor.matmul(
                        p[:],
                        nT[o][:, kt, :],
                        h2T["t"][:, kt, :],
                        start=(kt == 0),
                        stop=False,
                    )
                nc.tensor.matmul(
                    p[:], linh[o][:], ones_t[:, 0:EH], start=False, stop=False
                )
                nc.tensor.matmul(
                    p[:], ones_t[:, 0:128], lint[o][:], start=False, stop=True
                )
                copy(o, slab[:, o, :], p[:])
                # ship each o-slab as soon as it's ready; separate rings so
                # the two ~0.6us DMA issues don't serialize on one engine,
                # single_packet to avoid the 16-way completion wait
                ring = nc.scalar if o == 0 else nc.sync
                ring.dma_start(
                    slab_dst[:, o : o + 1, :],
                    slab[:, o : o + 1, :],
                    single_packet=True,
                )

    nc.compile()
    _cache[0] = nc
    return nc


def _pack(w, kt, dtype=ml_dtypes.bfloat16):
    """[kt*128, n] row-major -> [128, kt*n] partition-packed."""
    n = w.shape[1]
    return np.ascontiguousarray(
        w.reshape(kt, 128, n).transpose(1, 0, 2).reshape(128, kt * n).astype(dtype)
    )


def _prep_host(inputs):
    """Host-side input packing -> per-core in_maps + assembly info."""
    hs = np.asarray(inputs["hidden_states"], dtype=np.float32)
    start = np.asarray(inputs["entity_start"]).astype(np.int64)
    end = np.asarray(inputs["entity_end"]).astype(np.int64)
    label = np.asarray(inputs["entity_label"]).astype(np.int64)

    t = np.arange(T)
    mask = (
        (t[None, None, :] >= start[:, :, None]) & (t[None, None, :] < end[:, :, None])
    ).astype(np.float32)  # [B,E,T]
    counts = np.maximum(mask.sum(-1, keepdims=True), 1.0)
    masknT = (mask / counts).transpose(0, 2, 1)  # [B,T,E]

    def f32(x):
        return np.ascontiguousarray(np.asarray(x, dtype=np.float32))

    bf = ml_dtypes.bfloat16
    w_bil = f32(inputs["W_bil"])
    w_lin = f32(inputs["W_lin"])
    b_lin = f32(inputs["b_lin"])
    emb_all = f32(inputs["entity_emb_w"])

    # Wbil_0 | Wbil_1 | wlin cols interleaved per o as [head_o, tail_o]
    wb0 = _pack(w_bil[0], KT_H2, np.float32).reshape(128, KT_H2, H2)
    wb1 = _pack(w_bil[1], KT_H2, np.float32).reshape(128, KT_H2, H2)
    wl = np.stack(
        [
            w_lin[:H2, 0].reshape(KT_H2, 128).T,
            w_lin[H2:, 0].reshape(KT_H2, 128).T,
            w_lin[:H2, 1].reshape(KT_H2, 128).T,
            w_lin[H2:, 1].reshape(KT_H2, 128).T,
        ],
        axis=2,
    )  # [128, KT_H2, 4]
    # kt-major pack: per kt, [wb0 | wb1 | 4 wlin cols] -> one contiguous DMA
    wtl = np.ascontiguousarray(
        np.concatenate([wb0, wb1, wl], axis=2).reshape(128, -1).astype(bf)
    )

    smf = np.zeros((128, SF_COLS), np.float32)
    smf[:, 0:MT_H1] = f32(inputs["bh1"]).reshape(MT_H1, 128).T
    smf[:, MT_H1 : 2 * MT_H1] = f32(inputs["bt1"]).reshape(MT_H1, 128).T
    smf[:, 2 * MT_H1 : 2 * MT_H1 + MT_H2] = f32(inputs["bh2"]).reshape(MT_H2, 128).T
    smf[:, 2 * MT_H1 + MT_H2 : 2 * MT_H1 + 2 * MT_H2] = (
        f32(inputs["bt2"]).reshape(MT_H2, 128).T
    )
    smf[0, 2 * MT_H1 + 2 * MT_H2 : SF_COLS] = b_lin

    # layer-1 split: W1a = W1[:D] on device; C = emb @ W1[D:] folded on host
    w1a = {}
    cfold = {}
    for s, key in (("h", "Wh1"), ("t", "Wt1")):
        w1f = f32(inputs[key])
        w1a[s] = _pack(w1f[:D], KT_D)
        cfold[s] = (emb_all @ w1f[D:]).astype(np.float32)  # [NL, H1]

    shared = {
        "W1h": w1a["h"],
        "W1t": w1a["t"],
        "W2h": _pack(f32(inputs["Wh2"]), KT_H1),
        "W2t": _pack(f32(inputs["Wt2"]), KT_H1),
        "Wtl": wtl,
        "smf": smf,
    }

    in_maps = []
    for i in range(N_CORES):
        b, q = divmod(i, 4)
        r, c = divmod(q, 2)  # row-half, col-half of the table quadrant
        sel = np.concatenate(
            [np.arange(EH * r, EH * r + EH), np.arange(EH * c, EH * c + EH)]
        )  # [head subset | tail subset]
        mrot = np.ascontiguousarray(masknT[b][:, sel])
        lab = label[b][sel]  # [E]
        onehot = (lab[None, :] == np.arange(NL)[:, None]).astype(np.float32)  # [NL,E]
        pkg = np.zeros((NL, PKG_COLS), np.float32)
        pkg[:, 0:H1] = cfold["h"]
        pkg[:, H1 : 2 * H1] = cfold["t"]
        pkg[:, 2 * H1 : 2 * H1 + E] = onehot
        pkg[0, 2 * H1 + E :] = 1.0  # ones row
        mm = dict(shared)
        mm["hs"] = _pack(hs[b], KT_T)
        mm["masknT"] = _pack(mrot, KT_T)
        mm["pkg"] = pkg.astype(bf)
        in_maps.append(mm)

    head_idx = np.asarray(inputs["head_idx"]).astype(np.int64)
    tail_idx = np.asarray(inputs["tail_idx"]).astype(np.int64)
    return in_maps, (head_idx, tail_idx), 0


def kernel(**inputs) -> np.ndarray:
    in_maps, (head_idx, tail_idx), ni = _prep_host(inputs)
    nc = _build(ni)
    res = run_bass_kernel_spmd(nc, in_maps, list(range(N_CORES)))
    out = np.zeros((B, P, OUT), np.float32)
    for b in range(B):
        slabs = np.stack(
            [
                res.results[4 * b + q]["slab"].reshape(128, OUT, EH).astype(np.float32)
                for q in range(4)
            ]
        )  # [q, 128, OUT, EH]; q = 2*r + c
        e1, e2 = head_idx[b], tail_idx[b]
        q = 2 * (e1 // EH) + (e2 // EH)
        out[b] = slabs[q, e1 % EH, :, e2 % EH]
    return out
